# revision 13
# baseline (speedup 1.0000x reference)
"""Trainium2 Bass kernel for nn_CaptionDecoder (embedding -> masked LSTM -> vocab projection).

Sharding: the LSTM (B=32, S=64, H=512) is replicated on all 8 cores; the
vocab dimension of W_out/b_out is sharded 8-way (4000 per core). Each core
emits logits [S*B, 4000] bf16; the host concatenates along vocab -> f32.

Device dataflow per core (pipelined across 16 groups of 4 LSTM steps):
  - emb gathered+transposed on host -> emb_t [E, T] (pre-scaled x2048), streamed
  - xg = emb@W_x + b staged into SBUF (bf16) one group ahead through a small
    PSUM buffer; injected into four per-gate PSUM tiles [128, 512]
  - recurrence h_{t-1} @ W_h runs in fp8 DoubleRow (2x PE throughput): the
    transposed h ring is kept in fp8 (h pre-scaled x64), W_h in fp8 (x32),
    so gate pre-activations come out x2048 and the ScalarE activation's free
    input scale (1/2048) undoes it exactly
  - state update: bf16 c/h; c is blended in place with copy_predicated
    (Keras mask_zero), h via two scalar_tensor_tensor ops folding the mask
    and the x64 h scale
  - a second bf16 ring feeds the logits matmuls (keeps logits free of fp8
    input noise); W_out is pre-divided by 64 on host to undo the h scale
  - logits: ring block [128,128] stationary, W_out streamed, bias via K=1
    ones matmul, ScalarE/DVE copy to SBUF bf16, DMA out
  - filler work (xg staging, logits) is emitted at very low scheduler
    priority so it drips into PE idle gaps instead of delaying the chain;
    resident weight DMAs ride the idle GpSimd queue so they don't block the
    h0/c0 init path at startup.
"""

import sys
from contextlib import contextmanager

import numpy as np

if "/opt/trn_rl_repo" not in sys.path:
    sys.path.insert(0, "/opt/trn_rl_repo")

import concourse.bass as bass
import concourse.bacc as bacc
import concourse.mybir as mybir
import concourse.tile as tile
from concourse.bass_utils import run_bass_kernel_spmd
from concourse.masks import make_identity

VOCAB, EMBED, HIDDEN, CTX = 32000, 512, 512, 2048
B, S = 32, 64
G4 = 4 * HIDDEN  # 2048 gate width
NCORES = 8
VSH = VOCAB // NCORES  # 4000 vocab per core
P = 128
T = S * B  # 2048 tokens, t-major (tok = t*B + b)
NT = T // P  # 16 token tiles / groups
NK = HIDDEN // P  # 4 k-chunks over hidden/embed
NKC = CTX // P  # 16 k-chunks over context
NV = 8  # vocab slices per core
VS = VSH // NV  # 500 wide each
F32 = mybir.dt.float32
BF = mybir.dt.bfloat16
F8 = mybir.dt.float8e4

WS = 32.0  # W_h fp8 pre-scale
HS = 64.0  # h fp8 pre-scale
XS = WS * HS  # gate pre-activation scale (undone by activation input scale)

_CACHE: dict = {}

sig = mybir.ActivationFunctionType.Sigmoid
tanh = mybir.ActivationFunctionType.Tanh
MULT = mybir.AluOpType.mult
ADD = mybir.AluOpType.add


@contextmanager
def low_priority(tc, bump=1_000_000):
    """Emit instructions as if issued much later: the scheduler only picks
    them when nothing chain-critical is ready (pure filler work)."""
    p = tc.cur_priority
    tc.cur_priority = p + bump
    try:
        yield
    finally:
        tc.cur_priority = p


def _build_program() -> bass.Bass:
    nc = bacc.Bacc(None)

    ctx_d = nc.declare_dram_parameter("context_t", [CTX, B], BF, isOutput=False)
    embt_d = nc.declare_dram_parameter("emb_t", [EMBED, T], BF, isOutput=False)
    wih_d = nc.declare_dram_parameter("w_ih", [CTX, HIDDEN], BF, isOutput=False)
    wic_d = nc.declare_dram_parameter("w_ic", [CTX, HIDDEN], BF, isOutput=False)
    wx_d = nc.declare_dram_parameter("w_x", [EMBED, G4], BF, isOutput=False)
    wh3_d = nc.declare_dram_parameter("w_h3", [P, NK * G4], F8, isOutput=False)
    bg_d = nc.declare_dram_parameter("b_g", [G4], BF, isOutput=False)
    bih_d = nc.declare_dram_parameter("b_ih", [HIDDEN], BF, isOutput=False)
    bic_d = nc.declare_dram_parameter("b_ic", [HIDDEN], BF, isOutput=False)
    wout_d = nc.declare_dram_parameter("w_out", [HIDDEN, VSH], BF, isOutput=False)
    bout_d = nc.declare_dram_parameter("b_out", [VSH], BF, isOutput=False)
    mask_d = nc.declare_dram_parameter("maskf", [B, S], mybir.dt.uint8, isOutput=False)
    m64_d = nc.declare_dram_parameter("m64", [B, S], F32, isOutput=False)
    minv_d = nc.declare_dram_parameter("minv", [B, S], F32, isOutput=False)
    out_d = nc.declare_dram_parameter("logits", [T, VSH], BF, isOutput=True)

    with tile.TileContext(nc) as tc:
        with (
            tc.tile_pool(name="const", bufs=1) as cp,
            tc.tile_pool(name="stream", bufs=2) as sp,
            tc.tile_pool(name="embp", bufs=2) as ep,
            tc.tile_pool(name="xgp", bufs=2) as xp,
            tc.tile_pool(name="gates", bufs=2) as gp,
            tc.tile_pool(name="lout", bufs=3) as lp,
            tc.tile_pool(name="pz", bufs=1, space="PSUM") as pz,
            tc.tile_pool(name="pstage", bufs=1, space="PSUM") as psg,
            tc.tile_pool(name="pa", bufs=2, space="PSUM") as pa,
            tc.tile_pool(name="ptr", bufs=1, space="PSUM") as pt,
        ):
            # ---- resident constants / weights ----
            identb = cp.tile([P, P], BF, tag="identb", name="identb")
            make_identity(nc, identb[:])
            ones1 = cp.tile([1, P], BF, tag="ones1", name="ones1")
            nc.vector.memset(ones1[:], 1.0)

            ctx_sb = cp.tile([P, NKC * B], BF, tag="ctx", name="ctx")
            nc.sync.dma_start(
                out=ctx_sb[:].rearrange("p (k b) -> p k b", b=B),
                in_=ctx_d.rearrange("(k p) b -> p k b", p=P),
            )
            mask_sb = cp.tile([B, S], mybir.dt.uint8, tag="mask", name="mask")
            nc.sync.dma_start(out=mask_sb[:], in_=mask_d[:, :])
            m64_sb = cp.tile([B, S], F32, tag="m64", name="m64")
            nc.sync.dma_start(out=m64_sb[:], in_=m64_d[:, :])
            minv_sb = cp.tile([B, S], F32, tag="minv", name="minv")
            nc.sync.dma_start(out=minv_sb[:], in_=minv_d[:, :])
            bg_sb = cp.tile([1, G4], BF, tag="bg", name="bg")
            nc.sync.dma_start(out=bg_sb[:], in_=bg_d[None, :])
            bout_sb = cp.tile([1, VSH], BF, tag="bout", name="bout")
            nc.sync.dma_start(out=bout_sb[:], in_=bout_d[None, :])
            bih_sb = cp.tile([1, HIDDEN], BF, tag="bih", name="bih")
            nc.sync.dma_start(out=bih_sb[:], in_=bih_d[None, :])
            bic_sb = cp.tile([1, HIDDEN], BF, tag="bic", name="bic")
            nc.sync.dma_start(out=bic_sb[:], in_=bic_d[None, :])

            # resident weights load at low priority so the h0/c0 init path's
            # streaming DMAs win the queue at startup
            wh3_sb = cp.tile([P, NK * G4], F8, tag="wh3", name="wh3")
            wx_sb = []
            wout_sb = []
            with low_priority(tc):
                nc.sync.dma_start(out=wh3_sb[:], in_=wh3_d[:, :])
                for k in range(NK):
                    t_wx = cp.tile([P, G4], BF, tag=f"wx{k}", name=f"wx{k}")
                    nc.sync.dma_start(out=t_wx[:], in_=wx_d[k * P : (k + 1) * P, :])
                    wx_sb.append(t_wx)
                    t_wo = cp.tile([P, VSH], BF, tag=f"wout{k}", name=f"wout{k}")
                    nc.sync.dma_start(
                        out=t_wo[:], in_=wout_d[k * P : (k + 1) * P, :]
                    )
                    wout_sb.append(t_wo)
            wh3_v = wh3_sb[:].rearrange("p (o n) -> p o n", o=NK)

            # ---- embedding tiles (prefetched), staged xg in SBUF ----
            def load_embT(g):
                ts = []
                for k in range(NK):
                    et = ep.tile([P, P], BF, tag=f"embT{k}", name=f"embT{k}")
                    nc.sync.dma_start(
                        out=et[:],
                        in_=embt_d[k * P : (k + 1) * P, g * P : (g + 1) * P],
                    )
                    ts.append(et)
                return ts

            def stage_xg(embT):
                """xg = emb @ W_x + b for one group -> SBUF bf16 [128, 2048]."""
                xg = xp.tile([P, G4], BF, tag="xg", name="xg")
                for n in range(4):
                    ns = slice(n * HIDDEN, (n + 1) * HIDDEN)
                    ps_t = psg.tile([P, HIDDEN], F32, tag="xs", name="ps_t")
                    for k in range(NK):
                        nc.tensor.matmul(
                            out=ps_t[:],
                            lhsT=(embT[k][:]),
                            rhs=(wx_sb[k][:, ns]),
                            start=(k == 0),
                            stop=False,
                        )
                    nc.tensor.matmul(
                        out=ps_t[:],
                        lhsT=(ones1[:1, :]),
                        rhs=(bg_sb[:1, ns]),
                        start=False,
                        stop=True,
                    )
                    nc.any.tensor_copy(xg[:, ns], ps_t[:])
                return xg

            # ---- state tiles ----
            h_sb = cp.tile([B, HIDDEN], BF, tag="h", name="h")  # holds 64*h
            c_sb = cp.tile([B, HIDDEN], BF, tag="c", name="c")

            # h transpose rings: slot(t) = t % 8, cols (k*8 + slot)*32
            ring8 = cp.tile([P, NK * 8 * B], F8, tag="ring8", name="ring8")
            ringb = cp.tile([P, NK * 8 * B], BF, tag="ringb", name="ringb")

            def transpose_h(t):
                """PE-transpose h [32,512] into both rings' slot t%8."""
                slot = t % 8
                tp = pt.tile([P, P], BF, tag="tp", name="tp")
                for k in range(NK):
                    nc.tensor.transpose(
                        out=tp[:, k * B : (k + 1) * B],
                        in_=h_sb[:, k * P : (k + 1) * P],
                        identity=identb[:B, :B],
                    )
                src = tp[:].rearrange("p (k c) -> p k c", k=NK)
                dst8 = ring8[:].rearrange("p (k s c) -> p k s c", k=NK, s=8)[
                    :, :, slot, :
                ]
                nc.vector.tensor_copy(dst8, src)
                dstb = ringb[:].rearrange("p (k s c) -> p k s c", k=NK, s=8)[
                    :, :, slot, :
                ]
                nc.vector.tensor_copy(dstb, src)

            embT_cur = load_embT(0)
            embT_nxt = load_embT(1)

            # ---- initial state h0/c0 = tanh(context @ W + b) in gate tiles ----
            xz0_h = pz.tile([P, HIDDEN], F32, tag="xzg0", name="xz0_h")
            xz0_c = pz.tile([P, HIDDEN], F32, tag="xzg1", name="xz0_c")
            for w_dram, b_sb, dst in (
                (wih_d, bih_sb, xz0_h),
                (wic_d, bic_sb, xz0_c),
            ):
                for kc in range(NKC):
                    wt = sp.tile([P, HIDDEN], BF, tag="wstream", name="wstream")
                    nc.sync.dma_start(out=wt[:], in_=w_dram[kc * P : (kc + 1) * P, :])
                    nc.tensor.matmul(
                        out=dst[:B, :],
                        lhsT=(ctx_sb[:, kc * B : (kc + 1) * B]),
                        rhs=(wt[:]),
                        start=(kc == 0),
                        stop=False,
                    )
                nc.tensor.matmul(
                    out=dst[:B, :],
                    lhsT=(ones1[:1, :B]),
                    rhs=(b_sb[:1, :]),
                    start=False,
                    stop=True,
                )
            nc.scalar.activation(h_sb[:], xz0_h[:B, :], tanh)
            nc.vector.tensor_scalar_mul(h_sb[:], h_sb[:], HS)  # h_sb = 64*h0
            nc.scalar.activation(c_sb[:], xz0_c[:B, :], tanh)
            transpose_h(-1)  # h0 into slot 7

            xg_cur = stage_xg(embT_cur)

            def logits_group(g):
                """Vocab-sharded logits for token tile g from the bf16 ring."""
                half = (g % 2) * 4
                for v in range(NV):
                    vs = slice(v * VS, (v + 1) * VS)
                    pl = pa.tile([P, VS], F32, tag="pl", name="pl")
                    for k in range(NK):
                        cbase = (k * 8 + half) * B
                        nc.tensor.matmul(
                            out=pl[:],
                            lhsT=(ringb[:, cbase : cbase + 4 * B]),
                            rhs=(wout_sb[k][:, vs]),
                            start=(k == 0),
                            stop=False,
                        )
                    nc.tensor.matmul(
                        out=pl[:],
                        lhsT=(ones1[:1, :]),
                        rhs=(bout_sb[:1, vs]),
                        start=False,
                        stop=True,
                    )
                    lo = lp.tile([P, VS], BF, tag="lo", name="lo")
                    nc.any.tensor_copy(lo[:], pl[:])
                    nc.sync.dma_start(out=out_d[g * P : (g + 1) * P, vs], in_=lo[:])

            ring8_v = ring8[:].rearrange("p (k s c) -> p k s c", k=NK, s=8)

            # ---- main loop ----
            gate_tags = ["xzg0", "xzg1", "xzg2", "xzg3"]
            for g in range(NT):
                for s in range(4):
                    t = 4 * g + s
                    rows = slice(0, B)
                    slot_prev = (t - 1) % 8

                    # per-step inject: rows 32s of staged xg -> psum rows 0:32
                    # (DoubleRow requires dst partition base 0, so the 4 steps
                    # sequentially reuse the same per-gate psum rows)
                    xzg = []
                    for n in range(4):
                        zt = pz.tile(
                            [P, HIDDEN], F32, tag=gate_tags[n], name=f"xz{n}"
                        )
                        xzg.append(zt)
                    for n in (1, 0, 2, 3):
                        nc.tensor.matmul(
                            out=xzg[n][rows, :],
                            lhsT=(identb[:, B * s : B * (s + 1)]),
                            rhs=(xg_cur[:, n * HIDDEN : (n + 1) * HIDDEN]),
                            start=True,
                            stop=True,
                        )

                    # recurrence: z += (64h) @ (32W_h) in fp8 DoubleRow
                    for n in (1, 0, 2, 3):  # Keras gate order is i,f,g,o
                        ns = slice(n * HIDDEN, (n + 1) * HIDDEN)
                        for j in range(2):
                            nc.tensor.matmul(
                                out=xzg[n][rows, :],
                                lhsT=ring8_v[:, 2 * j : 2 * j + 2, slot_prev, :],
                                rhs=wh3_v[:, 2 * j : 2 * j + 2, ns],
                                perf_mode=mybir.MatmulPerfMode.DoubleRow,
                                start=False,
                                stop=False,
                                skip_group_check=True,
                            )

                    sig_f = gp.tile([B, HIDDEN], BF, tag="sig_f", name="sig_f")
                    sig_i = gp.tile([B, HIDDEN], BF, tag="sig_i", name="sig_i")
                    tanh_g = gp.tile([B, HIDDEN], BF, tag="tanh_g", name="tanh_g")
                    sig_o = gp.tile([B, HIDDEN], BF, tag="sig_o", name="sig_o")
                    nc.scalar.activation(sig_f[:], xzg[1][rows, :], sig, scale=1.0 / XS)
                    nc.scalar.activation(sig_i[:], xzg[0][rows, :], sig, scale=1.0 / XS)
                    nc.scalar.activation(
                        tanh_g[:], xzg[2][rows, :], tanh, scale=1.0 / XS
                    )
                    nc.scalar.activation(sig_o[:], xzg[3][rows, :], sig, scale=1.0 / XS)

                    t1 = gp.tile([B, HIDDEN], BF, tag="t1", name="t1")
                    t2 = gp.tile([B, HIDDEN], BF, tag="t2", name="t2")
                    c_new = gp.tile([B, HIDDEN], BF, tag="c_new", name="c_new")
                    nc.vector.tensor_mul(t1[:], sig_f[:], c_sb[:])
                    nc.vector.tensor_mul(t2[:], sig_i[:], tanh_g[:])
                    nc.vector.tensor_add(c_new[:], t1[:], t2[:])

                    m_bc = mask_sb[:, t : t + 1].to_broadcast([B, HIDDEN])
                    # masked (token==0) steps carry previous state; in-place blend
                    nc.vector.copy_predicated(c_sb[:], m_bc, c_new[:])

                    # h path uses pre-mask c_new: masked rows discard h_new anyway
                    tanh_c = gp.tile([B, HIDDEN], BF, tag="tanh_c", name="tanh_c")
                    nc.scalar.activation(tanh_c[:], c_new[:], tanh)
                    h_new = gp.tile([B, HIDDEN], BF, tag="h_new", name="h_new")
                    # h_new = (64*m) * tanh_c * sig_o;  h = (1-m)*h + h_new
                    nc.vector.scalar_tensor_tensor(
                        out=h_new[:],
                        in0=tanh_c[:],
                        scalar=m64_sb[:, t : t + 1],
                        in1=sig_o[:],
                        op0=MULT,
                        op1=MULT,
                    )
                    nc.vector.scalar_tensor_tensor(
                        out=h_sb[:],
                        in0=h_sb[:],
                        scalar=minv_sb[:, t : t + 1],
                        in1=h_new[:],
                        op0=MULT,
                        op1=ADD,
                    )

                    transpose_h(t)

                # filler work at very low priority: drips into PE idle gaps
                with low_priority(tc):
                    if g + 1 < NT:
                        xg_cur = stage_xg(embT_nxt)
                        if g + 2 < NT:
                            embT_nxt = load_embT(g + 2)
                    if g >= 1:
                        logits_group(g - 1)

            with low_priority(tc):
                logits_group(NT - 1)

    return nc


def _get_program() -> bass.Bass:
    if "nc" not in _CACHE:
        _CACHE["nc"] = _build_program()
    return _CACHE["nc"]


def prep_in_maps(inputs) -> list:
    import ml_dtypes

    bf16 = ml_dtypes.bfloat16
    fp8 = ml_dtypes.float8_e4m3
    tok = np.asarray(inputs["target_tokens"])
    ctx = np.asarray(inputs["context"], dtype=np.float32)
    emb_table = np.asarray(inputs["emb_table"], np.float32)
    w_h = np.asarray(inputs["W_h"], np.float32)
    w_out = np.asarray(inputs["W_out"], np.float32)
    b_out = np.asarray(inputs["b_out"], np.float32)

    mask = (tok != 0).astype(np.uint8)  # [B, S]
    tok_t = tok.T.reshape(-1).astype(np.int64)  # t*B + b token order
    emb_t = np.ascontiguousarray((emb_table[tok_t].T * XS).astype(bf16))  # [E, T]
    ctx_t = np.ascontiguousarray(ctx.T.astype(bf16))  # [CTX, B]

    w_h3 = np.clip(w_h * WS, -240.0, 240.0)  # [512, 2048] scaled
    w_h3 = np.ascontiguousarray(
        w_h3.reshape(NK, P, G4).transpose(1, 0, 2).reshape(P, NK * G4).astype(fp8)
    )

    shared = {
        "context_t": ctx_t,
        "emb_t": emb_t,
        "w_ih": np.ascontiguousarray(np.asarray(inputs["W_ih"]).astype(bf16)),
        "w_ic": np.ascontiguousarray(np.asarray(inputs["W_ic"]).astype(bf16)),
        "w_x": np.ascontiguousarray(np.asarray(inputs["W_x"]).astype(bf16)),
        "w_h3": w_h3,
        "b_g": np.ascontiguousarray((np.asarray(inputs["b"]) * XS).astype(bf16)),
        "b_ih": np.ascontiguousarray(np.asarray(inputs["b_ih"]).astype(bf16)),
        "b_ic": np.ascontiguousarray(np.asarray(inputs["b_ic"]).astype(bf16)),
        "maskf": np.ascontiguousarray(mask),
        "m64": np.ascontiguousarray(mask.astype(np.float32) * HS),
        "minv": np.ascontiguousarray(1.0 - mask.astype(np.float32)),
    }
    in_maps = []
    for j in range(NCORES):
        m = dict(shared)
        m["w_out"] = np.ascontiguousarray(
            (w_out[:, j * VSH : (j + 1) * VSH] / HS).astype(bf16)
        )
        m["b_out"] = np.ascontiguousarray(b_out[j * VSH : (j + 1) * VSH].astype(bf16))
        in_maps.append(m)
    return in_maps


def kernel(**inputs: np.ndarray) -> np.ndarray:
    in_maps = prep_in_maps(inputs)
    nc = _get_program()
    if not nc.is_finalized():
        nc.finalize()

    import os

    trace = bool(os.environ.get("CAPDEC_TRACE"))
    kw = {}
    if trace:
        kw["trace"] = True
        tdir = os.environ.get("CAPDEC_TRACE_DIR")
        if tdir:
            os.makedirs(tdir, exist_ok=True)
            kw["tmpdir"] = tdir
    bkr = run_bass_kernel_spmd(nc, in_maps, list(range(NCORES)), **kw)
    _CACHE["last_results"] = bkr
    res = bkr.results
    parts = [
        np.asarray(res[j]["logits"]).astype(np.float32).reshape(S, B, VSH)
        for j in range(NCORES)
    ]
    full = np.concatenate(parts, axis=-1)  # [S, B, VOCAB]
    return np.ascontiguousarray(full.transpose(1, 0, 2))


# revision 14
# speedup vs baseline: 1.0225x; 1.0225x over previous
"""Trainium2 Bass kernel for nn_CaptionDecoder (embedding -> masked LSTM -> vocab projection).

Sharding: the LSTM (B=32, S=64, H=512) is replicated on all 8 cores; the
vocab dimension of W_out/b_out is sharded 8-way (4000 per core). Each core
emits logits [S*B, 4000] bf16; the host concatenates along vocab -> f32.

Device dataflow per core (pipelined across 16 groups of 4 LSTM steps):
  - emb gathered+transposed on host -> emb_t [E, T] (pre-scaled x2048), streamed
  - xg = emb@W_x + b staged into SBUF (bf16) one group ahead through a small
    PSUM buffer; injected into four per-gate PSUM tiles [128, 512]
  - recurrence h_{t-1} @ W_h runs in fp8 DoubleRow (2x PE throughput): the
    transposed h ring is kept in fp8 (h pre-scaled x64), W_h in fp8 (x32),
    so gate pre-activations come out x2048 and the ScalarE activation's free
    input scale (1/2048) undoes it exactly
  - state update: bf16 c/h; c is blended in place with copy_predicated
    (Keras mask_zero), h via two scalar_tensor_tensor ops folding the mask
    and the x64 h scale
  - a second bf16 ring feeds the logits matmuls (keeps logits free of fp8
    input noise); W_out is pre-divided by 64 on host to undo the h scale
  - logits: ring block [128,128] stationary, W_out streamed, bias via K=1
    ones matmul, ScalarE/DVE copy to SBUF bf16, DMA out
  - filler work (xg staging, logits) is emitted at very low scheduler
    priority so it drips into PE idle gaps instead of delaying the chain;
    resident weight DMAs ride the idle GpSimd queue so they don't block the
    h0/c0 init path at startup.
"""

import sys
from contextlib import contextmanager

import numpy as np

if "/opt/trn_rl_repo" not in sys.path:
    sys.path.insert(0, "/opt/trn_rl_repo")

import concourse.bass as bass
import concourse.bacc as bacc
import concourse.mybir as mybir
import concourse.tile as tile
from concourse.bass_utils import run_bass_kernel_spmd
from concourse.masks import make_identity

VOCAB, EMBED, HIDDEN, CTX = 32000, 512, 512, 2048
B, S = 32, 64
G4 = 4 * HIDDEN  # 2048 gate width
NCORES = 8
VSH = VOCAB // NCORES  # 4000 vocab per core
P = 128
T = S * B  # 2048 tokens, t-major (tok = t*B + b)
NT = T // P  # 16 token tiles / groups
NK = HIDDEN // P  # 4 k-chunks over hidden/embed
NKC = CTX // P  # 16 k-chunks over context
NV = 8  # vocab slices per core
VS = VSH // NV  # 500 wide each
F32 = mybir.dt.float32
BF = mybir.dt.bfloat16
F8 = mybir.dt.float8e4

WS = 32.0  # W_h fp8 pre-scale
HS = 64.0  # h fp8 pre-scale
XS = WS * HS  # gate pre-activation scale (undone by activation input scale)

_CACHE: dict = {}

sig = mybir.ActivationFunctionType.Sigmoid
tanh = mybir.ActivationFunctionType.Tanh
MULT = mybir.AluOpType.mult
ADD = mybir.AluOpType.add


@contextmanager
def low_priority(tc, bump=1_000_000):
    """Emit instructions as if issued much later: the scheduler only picks
    them when nothing chain-critical is ready (pure filler work)."""
    p = tc.cur_priority
    tc.cur_priority = p + bump
    try:
        yield
    finally:
        tc.cur_priority = p


def _build_program() -> bass.Bass:
    nc = bacc.Bacc(None)

    ctx_d = nc.declare_dram_parameter("context_t", [CTX, B], BF, isOutput=False)
    embt_d = nc.declare_dram_parameter("emb_t", [EMBED, T], BF, isOutput=False)
    wih_d = nc.declare_dram_parameter("w_ih", [CTX, HIDDEN], BF, isOutput=False)
    wic_d = nc.declare_dram_parameter("w_ic", [CTX, HIDDEN], BF, isOutput=False)
    wx_d = nc.declare_dram_parameter("w_x", [EMBED, G4], BF, isOutput=False)
    wh3_d = nc.declare_dram_parameter("w_h3", [P, NK * G4], F8, isOutput=False)
    bg_d = nc.declare_dram_parameter("b_g", [G4], BF, isOutput=False)
    bih_d = nc.declare_dram_parameter("b_ih", [HIDDEN], BF, isOutput=False)
    bic_d = nc.declare_dram_parameter("b_ic", [HIDDEN], BF, isOutput=False)
    wout_d = nc.declare_dram_parameter("w_out", [HIDDEN, VSH], BF, isOutput=False)
    bout_d = nc.declare_dram_parameter("b_out", [VSH], BF, isOutput=False)
    mask_d = nc.declare_dram_parameter("maskf", [B, S], mybir.dt.uint8, isOutput=False)
    out_d = nc.declare_dram_parameter("logits", [T, VSH], BF, isOutput=True)

    with tile.TileContext(nc) as tc:
        with (
            tc.tile_pool(name="const", bufs=1) as cp,
            tc.tile_pool(name="stream", bufs=2) as sp,
            tc.tile_pool(name="embp", bufs=2) as ep,
            tc.tile_pool(name="xgp", bufs=2) as xp,
            tc.tile_pool(name="gates", bufs=2) as gp,
            tc.tile_pool(name="lout", bufs=3) as lp,
            tc.tile_pool(name="pz", bufs=1, space="PSUM") as pz,
            tc.tile_pool(name="pstage", bufs=1, space="PSUM") as psg,
            tc.tile_pool(name="pa", bufs=2, space="PSUM") as pa,
            tc.tile_pool(name="ptr", bufs=1, space="PSUM") as pt,
        ):
            # ---- resident constants / weights ----
            identb = cp.tile([P, P], BF, tag="identb", name="identb")
            make_identity(nc, identb[:])
            ones1 = cp.tile([1, P], BF, tag="ones1", name="ones1")
            nc.vector.memset(ones1[:], 1.0)

            ctx_sb = cp.tile([P, NKC * B], BF, tag="ctx", name="ctx")
            nc.sync.dma_start(
                out=ctx_sb[:].rearrange("p (k b) -> p k b", b=B),
                in_=ctx_d.rearrange("(k p) b -> p k b", p=P),
            )
            mask_sb = cp.tile([B, S], mybir.dt.uint8, tag="mask", name="mask")
            nc.sync.dma_start(out=mask_sb[:], in_=mask_d[:, :])
            bg_sb = cp.tile([1, G4], BF, tag="bg", name="bg")
            nc.sync.dma_start(out=bg_sb[:], in_=bg_d[None, :])
            bout_sb = cp.tile([1, VSH], BF, tag="bout", name="bout")
            nc.sync.dma_start(out=bout_sb[:], in_=bout_d[None, :])
            bih_sb = cp.tile([1, HIDDEN], BF, tag="bih", name="bih")
            nc.sync.dma_start(out=bih_sb[:], in_=bih_d[None, :])
            bic_sb = cp.tile([1, HIDDEN], BF, tag="bic", name="bic")
            nc.sync.dma_start(out=bic_sb[:], in_=bic_d[None, :])

            # resident weights load at low priority so the h0/c0 init path's
            # streaming DMAs win the queue at startup
            wh3_sb = cp.tile([P, NK * G4], F8, tag="wh3", name="wh3")
            wx_sb = []
            wout_sb = []
            with low_priority(tc):
                nc.sync.dma_start(out=wh3_sb[:], in_=wh3_d[:, :])
                for k in range(NK):
                    t_wx = cp.tile([P, G4], BF, tag=f"wx{k}", name=f"wx{k}")
                    nc.sync.dma_start(out=t_wx[:], in_=wx_d[k * P : (k + 1) * P, :])
                    wx_sb.append(t_wx)
                    t_wo = cp.tile([P, VSH], BF, tag=f"wout{k}", name=f"wout{k}")
                    nc.sync.dma_start(
                        out=t_wo[:], in_=wout_d[k * P : (k + 1) * P, :]
                    )
                    wout_sb.append(t_wo)
            wh3_v = wh3_sb[:].rearrange("p (o n) -> p o n", o=NK)

            # ---- embedding tiles (prefetched), staged xg in SBUF ----
            def load_embT(g):
                ts = []
                for k in range(NK):
                    et = ep.tile([P, P], BF, tag=f"embT{k}", name=f"embT{k}")
                    nc.sync.dma_start(
                        out=et[:],
                        in_=embt_d[k * P : (k + 1) * P, g * P : (g + 1) * P],
                    )
                    ts.append(et)
                return ts

            def stage_xg(embT):
                """xg = emb @ W_x + b for one group -> SBUF bf16 [128, 2048]."""
                xg = xp.tile([P, G4], BF, tag="xg", name="xg")
                for n in range(4):
                    ns = slice(n * HIDDEN, (n + 1) * HIDDEN)
                    ps_t = psg.tile([P, HIDDEN], F32, tag="xs", name="ps_t")
                    for k in range(NK):
                        nc.tensor.matmul(
                            out=ps_t[:],
                            lhsT=(embT[k][:]),
                            rhs=(wx_sb[k][:, ns]),
                            start=(k == 0),
                            stop=False,
                        )
                    nc.tensor.matmul(
                        out=ps_t[:],
                        lhsT=(ones1[:1, :]),
                        rhs=(bg_sb[:1, ns]),
                        start=False,
                        stop=True,
                    )
                    nc.vector.tensor_copy(xg[:, ns], ps_t[:])
                return xg

            # ---- state tiles ----
            h_sb = cp.tile([B, HIDDEN], BF, tag="h", name="h")
            c_sb = cp.tile([B, HIDDEN], BF, tag="c", name="c")

            # h transpose rings: slot(t) = t % 8, cols (k*8 + slot)*32
            ring8 = cp.tile([P, NK * 8 * B], F8, tag="ring8", name="ring8")
            ringb = cp.tile([P, NK * 8 * B], BF, tag="ringb", name="ringb")

            def transpose_h(t):
                """PE-transpose h [32,512] into both rings' slot t%8."""
                slot = t % 8
                tp = pt.tile([P, P], BF, tag="tp", name="tp")
                for k in range(NK):
                    nc.tensor.transpose(
                        out=tp[:, k * B : (k + 1) * B],
                        in_=h_sb[:, k * P : (k + 1) * P],
                        identity=identb[:B, :B],
                    )
                src = tp[:].rearrange("p (k c) -> p k c", k=NK)
                dst8 = ring8[:].rearrange("p (k s c) -> p k s c", k=NK, s=8)[
                    :, :, slot, :
                ]
                nc.vector.tensor_scalar_mul(dst8, src, HS)
                dstb = ringb[:].rearrange("p (k s c) -> p k s c", k=NK, s=8)[
                    :, :, slot, :
                ]
                nc.vector.tensor_copy(dstb, src)

            embT_cur = load_embT(0)
            embT_nxt = load_embT(1)

            # ---- initial state h0/c0 = tanh(context @ W + b) in gate tiles ----
            xz0_h = pz.tile([P, HIDDEN], F32, tag="xzg0", name="xz0_h")
            xz0_c = pz.tile([P, HIDDEN], F32, tag="xzg1", name="xz0_c")
            for w_dram, b_sb, dst in (
                (wih_d, bih_sb, xz0_h),
                (wic_d, bic_sb, xz0_c),
            ):
                for kc in range(NKC):
                    wt = sp.tile([P, HIDDEN], BF, tag="wstream", name="wstream")
                    nc.sync.dma_start(out=wt[:], in_=w_dram[kc * P : (kc + 1) * P, :])
                    nc.tensor.matmul(
                        out=dst[:B, :],
                        lhsT=(ctx_sb[:, kc * B : (kc + 1) * B]),
                        rhs=(wt[:]),
                        start=(kc == 0),
                        stop=False,
                    )
                nc.tensor.matmul(
                    out=dst[:B, :],
                    lhsT=(ones1[:1, :B]),
                    rhs=(b_sb[:1, :]),
                    start=False,
                    stop=True,
                )
            nc.scalar.activation(h_sb[:], xz0_h[:B, :], tanh)
            nc.scalar.activation(c_sb[:], xz0_c[:B, :], tanh)
            transpose_h(-1)  # h0 into slot 7

            xg_cur = stage_xg(embT_cur)

            def logits_group(g):
                """Vocab-sharded logits for token tile g from the bf16 ring."""
                half = (g % 2) * 4
                for v in range(NV):
                    vs = slice(v * VS, (v + 1) * VS)
                    pl = pa.tile([P, VS], F32, tag="pl", name="pl")
                    for k in range(NK):
                        cbase = (k * 8 + half) * B
                        nc.tensor.matmul(
                            out=pl[:],
                            lhsT=(ringb[:, cbase : cbase + 4 * B]),
                            rhs=(wout_sb[k][:, vs]),
                            start=(k == 0),
                            stop=False,
                        )
                    nc.tensor.matmul(
                        out=pl[:],
                        lhsT=(ones1[:1, :]),
                        rhs=(bout_sb[:1, vs]),
                        start=False,
                        stop=True,
                    )
                    lo = lp.tile([P, VS], BF, tag="lo", name="lo")
                    if v % 2 == 0:
                        nc.scalar.copy(lo[:], pl[:])
                    else:
                        nc.vector.tensor_copy(lo[:], pl[:])
                    nc.sync.dma_start(out=out_d[g * P : (g + 1) * P, vs], in_=lo[:])

            ring8_v = ring8[:].rearrange("p (k s c) -> p k s c", k=NK, s=8)

            # ---- main loop ----
            gate_tags = ["xzg0", "xzg1", "xzg2", "xzg3"]
            for g in range(NT):
                for s in range(4):
                    t = 4 * g + s
                    rows = slice(0, B)
                    slot_prev = (t - 1) % 8

                    # per-step inject: rows 32s of staged xg -> psum rows 0:32
                    # (DoubleRow requires dst partition base 0, so the 4 steps
                    # sequentially reuse the same per-gate psum rows)
                    xzg = []
                    for n in range(4):
                        zt = pz.tile(
                            [P, HIDDEN], F32, tag=gate_tags[n], name=f"xz{n}"
                        )
                        xzg.append(zt)
                    for n in (1, 0, 2, 3):
                        nc.tensor.matmul(
                            out=xzg[n][rows, :],
                            lhsT=(identb[:, B * s : B * (s + 1)]),
                            rhs=(xg_cur[:, n * HIDDEN : (n + 1) * HIDDEN]),
                            start=True,
                            stop=True,
                        )

                    # recurrence: z += (64h) @ (32W_h) in fp8 DoubleRow
                    for n in (1, 0, 2, 3):  # Keras gate order is i,f,g,o
                        ns = slice(n * HIDDEN, (n + 1) * HIDDEN)
                        for j in range(2):
                            nc.tensor.matmul(
                                out=xzg[n][rows, :],
                                lhsT=ring8_v[:, 2 * j : 2 * j + 2, slot_prev, :],
                                rhs=wh3_v[:, 2 * j : 2 * j + 2, ns],
                                perf_mode=mybir.MatmulPerfMode.DoubleRow,
                                start=False,
                                stop=False,
                                skip_group_check=True,
                            )

                    sig_f = gp.tile([B, HIDDEN], BF, tag="sig_f", name="sig_f")
                    sig_i = gp.tile([B, HIDDEN], BF, tag="sig_i", name="sig_i")
                    tanh_g = gp.tile([B, HIDDEN], BF, tag="tanh_g", name="tanh_g")
                    sig_o = gp.tile([B, HIDDEN], BF, tag="sig_o", name="sig_o")
                    nc.scalar.activation(sig_f[:], xzg[1][rows, :], sig, scale=1.0 / XS)
                    nc.scalar.activation(sig_i[:], xzg[0][rows, :], sig, scale=1.0 / XS)
                    nc.scalar.activation(
                        tanh_g[:], xzg[2][rows, :], tanh, scale=1.0 / XS
                    )
                    nc.scalar.activation(sig_o[:], xzg[3][rows, :], sig, scale=1.0 / XS)

                    t1 = gp.tile([B, HIDDEN], BF, tag="t1", name="t1")
                    t2 = gp.tile([B, HIDDEN], BF, tag="t2", name="t2")
                    c_new = gp.tile([B, HIDDEN], BF, tag="c_new", name="c_new")
                    nc.vector.tensor_mul(t1[:], sig_f[:], c_sb[:])
                    nc.vector.tensor_mul(t2[:], sig_i[:], tanh_g[:])
                    nc.vector.tensor_add(c_new[:], t1[:], t2[:])

                    m_bc = mask_sb[:, t : t + 1].to_broadcast([B, HIDDEN])
                    # masked (token==0) steps carry previous state; in-place blend
                    nc.vector.copy_predicated(c_sb[:], m_bc, c_new[:])

                    # h path uses pre-mask c_new: masked rows discard h_new anyway
                    tanh_c = gp.tile([B, HIDDEN], BF, tag="tanh_c", name="tanh_c")
                    nc.scalar.activation(tanh_c[:], c_new[:], tanh)
                    h_new = gp.tile([B, HIDDEN], BF, tag="h_new", name="h_new")
                    nc.vector.tensor_mul(h_new[:], sig_o[:], tanh_c[:])
                    nc.vector.copy_predicated(h_sb[:], m_bc, h_new[:])

                    transpose_h(t)

                # filler work at very low priority: drips into PE idle gaps
                with low_priority(tc):
                    if g + 1 < NT:
                        xg_cur = stage_xg(embT_nxt)
                        if g + 2 < NT:
                            embT_nxt = load_embT(g + 2)
                    if g >= 1:
                        logits_group(g - 1)

            with low_priority(tc):
                logits_group(NT - 1)

    return nc


def _get_program() -> bass.Bass:
    if "nc" not in _CACHE:
        _CACHE["nc"] = _build_program()
    return _CACHE["nc"]


def prep_in_maps(inputs) -> list:
    import ml_dtypes

    bf16 = ml_dtypes.bfloat16
    fp8 = ml_dtypes.float8_e4m3
    tok = np.asarray(inputs["target_tokens"])
    ctx = np.asarray(inputs["context"], dtype=np.float32)
    emb_table = np.asarray(inputs["emb_table"], np.float32)
    w_h = np.asarray(inputs["W_h"], np.float32)
    w_out = np.asarray(inputs["W_out"], np.float32)
    b_out = np.asarray(inputs["b_out"], np.float32)

    mask = (tok != 0).astype(np.uint8)  # [B, S]
    tok_t = tok.T.reshape(-1).astype(np.int64)  # t*B + b token order
    emb_t = np.ascontiguousarray((emb_table[tok_t].T * XS).astype(bf16))  # [E, T]
    ctx_t = np.ascontiguousarray(ctx.T.astype(bf16))  # [CTX, B]

    w_h3 = np.clip(w_h * WS, -240.0, 240.0)  # [512, 2048] scaled
    w_h3 = np.ascontiguousarray(
        w_h3.reshape(NK, P, G4).transpose(1, 0, 2).reshape(P, NK * G4).astype(fp8)
    )

    shared = {
        "context_t": ctx_t,
        "emb_t": emb_t,
        "w_ih": np.ascontiguousarray(np.asarray(inputs["W_ih"]).astype(bf16)),
        "w_ic": np.ascontiguousarray(np.asarray(inputs["W_ic"]).astype(bf16)),
        "w_x": np.ascontiguousarray(np.asarray(inputs["W_x"]).astype(bf16)),
        "w_h3": w_h3,
        "b_g": np.ascontiguousarray((np.asarray(inputs["b"]) * XS).astype(bf16)),
        "b_ih": np.ascontiguousarray(np.asarray(inputs["b_ih"]).astype(bf16)),
        "b_ic": np.ascontiguousarray(np.asarray(inputs["b_ic"]).astype(bf16)),
        "maskf": np.ascontiguousarray(mask),
    }
    in_maps = []
    for j in range(NCORES):
        m = dict(shared)
        m["w_out"] = np.ascontiguousarray(
            w_out[:, j * VSH : (j + 1) * VSH].astype(bf16)
        )
        m["b_out"] = np.ascontiguousarray(b_out[j * VSH : (j + 1) * VSH].astype(bf16))
        in_maps.append(m)
    return in_maps


def kernel(**inputs: np.ndarray) -> np.ndarray:
    in_maps = prep_in_maps(inputs)
    nc = _get_program()
    if not nc.is_finalized():
        nc.finalize()

    import os

    trace = bool(os.environ.get("CAPDEC_TRACE"))
    kw = {}
    if trace:
        kw["trace"] = True
        tdir = os.environ.get("CAPDEC_TRACE_DIR")
        if tdir:
            os.makedirs(tdir, exist_ok=True)
            kw["tmpdir"] = tdir
    bkr = run_bass_kernel_spmd(nc, in_maps, list(range(NCORES)), **kw)
    _CACHE["last_results"] = bkr
    res = bkr.results
    parts = [
        np.asarray(res[j]["logits"]).astype(np.float32).reshape(S, B, VSH)
        for j in range(NCORES)
    ]
    full = np.concatenate(parts, axis=-1)  # [S, B, VOCAB]
    return np.ascontiguousarray(full.transpose(1, 0, 2))


# revision 15
# speedup vs baseline: 1.0718x; 1.0482x over previous
"""Trainium2 Bass kernel for nn_CaptionDecoder (embedding -> masked LSTM -> vocab projection).

Sharding: the LSTM (B=32, S=64, H=512) is replicated on all 8 cores; the
vocab dimension of W_out/b_out is sharded 8-way (4000 per core). Each core
emits logits [S*B, 4000] bf16; the host concatenates along vocab -> f32.

Device dataflow per core (pipelined across 16 groups of 4 LSTM steps):
  - emb gathered+transposed on host -> emb_t [E, T] (pre-scaled x2048), streamed
  - xg = emb@W_x + b staged into SBUF (bf16) one group ahead through a small
    PSUM buffer; injected into four per-gate PSUM tiles [128, 512]
  - recurrence h_{t-1} @ W_h runs in fp8 DoubleRow (2x PE throughput): the
    transposed h ring is kept in fp8 (h pre-scaled x64), W_h in fp8 (x32),
    so gate pre-activations come out x2048 and the ScalarE activation's free
    input scale (1/2048) undoes it exactly
  - state update: bf16 c/h; c is blended in place with copy_predicated
    (Keras mask_zero), h via two scalar_tensor_tensor ops folding the mask
    and the x64 h scale
  - a second bf16 ring feeds the logits matmuls (keeps logits free of fp8
    input noise); W_out is pre-divided by 64 on host to undo the h scale
  - logits: ring block [128,128] stationary, W_out streamed, bias via K=1
    ones matmul, ScalarE/DVE copy to SBUF bf16, DMA out
  - filler work (xg staging, logits) is emitted at very low scheduler
    priority so it drips into PE idle gaps instead of delaying the chain;
    resident weight DMAs ride the idle GpSimd queue so they don't block the
    h0/c0 init path at startup.
"""

import sys
from contextlib import contextmanager

import numpy as np

if "/opt/trn_rl_repo" not in sys.path:
    sys.path.insert(0, "/opt/trn_rl_repo")

import concourse.bass as bass
import concourse.bacc as bacc
import concourse.mybir as mybir
import concourse.tile as tile
from concourse.bass_utils import run_bass_kernel_spmd
from concourse.masks import make_identity

VOCAB, EMBED, HIDDEN, CTX = 32000, 512, 512, 2048
B, S = 32, 64
G4 = 4 * HIDDEN  # 2048 gate width
NCORES = 8
VSH = VOCAB // NCORES  # 4000 vocab per core
P = 128
T = S * B  # 2048 tokens, t-major (tok = t*B + b)
NT = T // P  # 16 token tiles / groups
NK = HIDDEN // P  # 4 k-chunks over hidden/embed
NKC = CTX // P  # 16 k-chunks over context
NV = 8  # vocab slices per core
VS = VSH // NV  # 500 wide each
F32 = mybir.dt.float32
BF = mybir.dt.bfloat16
F8 = mybir.dt.float8e4

WS = 32.0  # W_h fp8 pre-scale
HS = 64.0  # h fp8 pre-scale
XS = WS * HS  # gate pre-activation scale (undone by activation input scale)

_CACHE: dict = {}

sig = mybir.ActivationFunctionType.Sigmoid
tanh = mybir.ActivationFunctionType.Tanh
MULT = mybir.AluOpType.mult
ADD = mybir.AluOpType.add


@contextmanager
def low_priority(tc, bump=1_000_000):
    """Emit instructions as if issued much later: the scheduler only picks
    them when nothing chain-critical is ready (pure filler work)."""
    p = tc.cur_priority
    tc.cur_priority = p + bump
    try:
        yield
    finally:
        tc.cur_priority = p


def _build_program() -> bass.Bass:
    nc = bacc.Bacc(None)

    ctx_d = nc.declare_dram_parameter("context_t", [CTX, B], BF, isOutput=False)
    embt_d = nc.declare_dram_parameter("emb_t", [EMBED, T], BF, isOutput=False)
    wih_d = nc.declare_dram_parameter("w_ih", [CTX, HIDDEN], BF, isOutput=False)
    wic_d = nc.declare_dram_parameter("w_ic", [CTX, HIDDEN], BF, isOutput=False)
    wx_d = nc.declare_dram_parameter("w_x", [EMBED, G4], BF, isOutput=False)
    wh3_d = nc.declare_dram_parameter("w_h3", [P, NK * G4], F8, isOutput=False)
    bg_d = nc.declare_dram_parameter("b_g", [G4], BF, isOutput=False)
    bih_d = nc.declare_dram_parameter("b_ih", [HIDDEN], BF, isOutput=False)
    bic_d = nc.declare_dram_parameter("b_ic", [HIDDEN], BF, isOutput=False)
    wout_d = nc.declare_dram_parameter("w_out", [HIDDEN, VSH], BF, isOutput=False)
    bout_d = nc.declare_dram_parameter("b_out", [VSH], BF, isOutput=False)
    mask_d = nc.declare_dram_parameter("maskf", [B, S], mybir.dt.uint8, isOutput=False)
    out_d = nc.declare_dram_parameter("logits", [T, VSH], BF, isOutput=True)

    with tile.TileContext(nc) as tc:
        with (
            tc.tile_pool(name="const", bufs=1) as cp,
            tc.tile_pool(name="stream", bufs=2) as sp,
            tc.tile_pool(name="embp", bufs=2) as ep,
            tc.tile_pool(name="xgp", bufs=2) as xp,
            tc.tile_pool(name="gates", bufs=2) as gp,
            tc.tile_pool(name="lout", bufs=3) as lp,
            tc.tile_pool(name="pz", bufs=1, space="PSUM") as pz,
            tc.tile_pool(name="pstage", bufs=1, space="PSUM") as psg,
            tc.tile_pool(name="pa", bufs=2, space="PSUM") as pa,
            tc.tile_pool(name="ptr", bufs=1, space="PSUM") as pt,
        ):
            # ---- resident constants / weights ----
            identb = cp.tile([P, P], BF, tag="identb", name="identb")
            make_identity(nc, identb[:])
            ones1 = cp.tile([1, P], BF, tag="ones1", name="ones1")
            nc.vector.memset(ones1[:], 1.0)

            ctx_sb = cp.tile([P, NKC * B], BF, tag="ctx", name="ctx")
            nc.sync.dma_start(
                out=ctx_sb[:].rearrange("p (k b) -> p k b", b=B),
                in_=ctx_d.rearrange("(k p) b -> p k b", p=P),
            )
            mask_sb = cp.tile([B, S], mybir.dt.uint8, tag="mask", name="mask")
            nc.sync.dma_start(out=mask_sb[:], in_=mask_d[:, :])
            bg_sb = cp.tile([1, G4], BF, tag="bg", name="bg")
            nc.sync.dma_start(out=bg_sb[:], in_=bg_d[None, :])
            bout_sb = cp.tile([1, VSH], BF, tag="bout", name="bout")
            nc.sync.dma_start(out=bout_sb[:], in_=bout_d[None, :])
            bih_sb = cp.tile([1, HIDDEN], BF, tag="bih", name="bih")
            nc.sync.dma_start(out=bih_sb[:], in_=bih_d[None, :])
            bic_sb = cp.tile([1, HIDDEN], BF, tag="bic", name="bic")
            nc.sync.dma_start(out=bic_sb[:], in_=bic_d[None, :])

            # resident weights load at low priority so the h0/c0 init path's
            # streaming DMAs win the queue at startup
            wh3_sb = cp.tile([P, NK * G4], F8, tag="wh3", name="wh3")
            wx_sb = []
            wout_sb = []
            with low_priority(tc):
                nc.sync.dma_start(out=wh3_sb[:], in_=wh3_d[:, :])
                for k in range(NK):
                    t_wx = cp.tile([P, G4], BF, tag=f"wx{k}", name=f"wx{k}")
                    nc.sync.dma_start(out=t_wx[:], in_=wx_d[k * P : (k + 1) * P, :])
                    wx_sb.append(t_wx)
                    t_wo = cp.tile([P, VSH], BF, tag=f"wout{k}", name=f"wout{k}")
                    nc.sync.dma_start(
                        out=t_wo[:], in_=wout_d[k * P : (k + 1) * P, :]
                    )
                    wout_sb.append(t_wo)
            wh3_v = wh3_sb[:].rearrange("p (o n) -> p o n", o=NK)

            # ---- embedding tiles (prefetched), staged xg in SBUF ----
            def load_embT(g):
                ts = []
                for k in range(NK):
                    et = ep.tile([P, P], BF, tag=f"embT{k}", name=f"embT{k}")
                    nc.sync.dma_start(
                        out=et[:],
                        in_=embt_d[k * P : (k + 1) * P, g * P : (g + 1) * P],
                    )
                    ts.append(et)
                return ts

            def stage_xg(embT):
                """xg = emb @ W_x + b for one group -> SBUF bf16 [128, 2048]."""
                xg = xp.tile([P, G4], BF, tag="xg", name="xg")
                for n in range(4):
                    ns = slice(n * HIDDEN, (n + 1) * HIDDEN)
                    ps_t = psg.tile([P, HIDDEN], F32, tag="xs", name="ps_t")
                    for k in range(NK):
                        nc.tensor.matmul(
                            out=ps_t[:],
                            lhsT=(embT[k][:]),
                            rhs=(wx_sb[k][:, ns]),
                            start=(k == 0),
                            stop=False,
                        )
                    nc.tensor.matmul(
                        out=ps_t[:],
                        lhsT=(ones1[:1, :]),
                        rhs=(bg_sb[:1, ns]),
                        start=False,
                        stop=True,
                    )
                    nc.scalar.copy(xg[:, ns], ps_t[:])
                return xg

            # ---- state tiles ----
            h_sb = cp.tile([B, HIDDEN], BF, tag="h", name="h")
            c_sb = cp.tile([B, HIDDEN], BF, tag="c", name="c")

            # h transpose rings: slot(t) = t % 8, cols (k*8 + slot)*32
            ring8 = cp.tile([P, NK * 8 * B], F8, tag="ring8", name="ring8")
            ringb = cp.tile([P, NK * 8 * B], BF, tag="ringb", name="ringb")

            def transpose_h(t):
                """PE-transpose h [32,512] into both rings' slot t%8."""
                slot = t % 8
                tp = pt.tile([P, P], BF, tag="tp", name="tp")
                for k in range(NK):
                    nc.tensor.transpose(
                        out=tp[:, k * B : (k + 1) * B],
                        in_=h_sb[:, k * P : (k + 1) * P],
                        identity=identb[:B, :B],
                    )
                src = tp[:].rearrange("p (k c) -> p k c", k=NK)
                dst8 = ring8[:].rearrange("p (k s c) -> p k s c", k=NK, s=8)[
                    :, :, slot, :
                ]
                nc.vector.tensor_scalar_mul(dst8, src, HS)
                dstb = ringb[:].rearrange("p (k s c) -> p k s c", k=NK, s=8)[
                    :, :, slot, :
                ]
                nc.vector.tensor_copy(dstb, src)

            embT_cur = load_embT(0)
            embT_nxt = load_embT(1)

            # ---- initial state h0/c0 = tanh(context @ W + b) in gate tiles ----
            xz0_h = pz.tile([P, HIDDEN], F32, tag="xzg0", name="xz0_h")
            xz0_c = pz.tile([P, HIDDEN], F32, tag="xzg1", name="xz0_c")
            for w_dram, b_sb, dst in (
                (wih_d, bih_sb, xz0_h),
                (wic_d, bic_sb, xz0_c),
            ):
                for kc in range(NKC):
                    wt = sp.tile([P, HIDDEN], BF, tag="wstream", name="wstream")
                    nc.sync.dma_start(out=wt[:], in_=w_dram[kc * P : (kc + 1) * P, :])
                    nc.tensor.matmul(
                        out=dst[:B, :],
                        lhsT=(ctx_sb[:, kc * B : (kc + 1) * B]),
                        rhs=(wt[:]),
                        start=(kc == 0),
                        stop=False,
                    )
                nc.tensor.matmul(
                    out=dst[:B, :],
                    lhsT=(ones1[:1, :B]),
                    rhs=(b_sb[:1, :]),
                    start=False,
                    stop=True,
                )
            nc.scalar.activation(h_sb[:], xz0_h[:B, :], tanh)
            nc.scalar.activation(c_sb[:], xz0_c[:B, :], tanh)
            transpose_h(-1)  # h0 into slot 7

            xg_cur = stage_xg(embT_cur)

            def logits_group(g):
                """Vocab-sharded logits for token tile g from the bf16 ring."""
                half = (g % 2) * 4
                for v in range(NV):
                    vs = slice(v * VS, (v + 1) * VS)
                    pl = pa.tile([P, VS], F32, tag="pl", name="pl")
                    for k in range(NK):
                        cbase = (k * 8 + half) * B
                        nc.tensor.matmul(
                            out=pl[:],
                            lhsT=(ringb[:, cbase : cbase + 4 * B]),
                            rhs=(wout_sb[k][:, vs]),
                            start=(k == 0),
                            stop=False,
                        )
                    nc.tensor.matmul(
                        out=pl[:],
                        lhsT=(ones1[:1, :]),
                        rhs=(bout_sb[:1, vs]),
                        start=False,
                        stop=True,
                    )
                    lo = lp.tile([P, VS], BF, tag="lo", name="lo")
                    nc.scalar.copy(lo[:], pl[:])
                    nc.sync.dma_start(out=out_d[g * P : (g + 1) * P, vs], in_=lo[:])

            ring8_v = ring8[:].rearrange("p (k s c) -> p k s c", k=NK, s=8)

            # ---- main loop ----
            gate_tags = ["xzg0", "xzg1", "xzg2", "xzg3"]
            for g in range(NT):
                for s in range(4):
                    t = 4 * g + s
                    rows = slice(0, B)
                    slot_prev = (t - 1) % 8

                    # per-step inject: rows 32s of staged xg -> psum rows 0:32
                    # (DoubleRow requires dst partition base 0, so the 4 steps
                    # sequentially reuse the same per-gate psum rows)
                    xzg = []
                    for n in range(4):
                        zt = pz.tile(
                            [P, HIDDEN], F32, tag=gate_tags[n], name=f"xz{n}"
                        )
                        xzg.append(zt)
                    for n in (1, 0, 2, 3):
                        nc.tensor.matmul(
                            out=xzg[n][rows, :],
                            lhsT=(identb[:, B * s : B * (s + 1)]),
                            rhs=(xg_cur[:, n * HIDDEN : (n + 1) * HIDDEN]),
                            start=True,
                            stop=True,
                        )

                    # recurrence: z += (64h) @ (32W_h) in fp8 DoubleRow
                    for n in (1, 0, 2, 3):  # Keras gate order is i,f,g,o
                        ns = slice(n * HIDDEN, (n + 1) * HIDDEN)
                        for j in range(2):
                            nc.tensor.matmul(
                                out=xzg[n][rows, :],
                                lhsT=ring8_v[:, 2 * j : 2 * j + 2, slot_prev, :],
                                rhs=wh3_v[:, 2 * j : 2 * j + 2, ns],
                                perf_mode=mybir.MatmulPerfMode.DoubleRow,
                                start=False,
                                stop=False,
                                skip_group_check=True,
                            )

                    sig_f = gp.tile([B, HIDDEN], BF, tag="sig_f", name="sig_f")
                    sig_i = gp.tile([B, HIDDEN], BF, tag="sig_i", name="sig_i")
                    tanh_g = gp.tile([B, HIDDEN], BF, tag="tanh_g", name="tanh_g")
                    sig_o = gp.tile([B, HIDDEN], BF, tag="sig_o", name="sig_o")
                    nc.scalar.activation(sig_f[:], xzg[1][rows, :], sig, scale=1.0 / XS)
                    nc.scalar.activation(sig_i[:], xzg[0][rows, :], sig, scale=1.0 / XS)
                    nc.scalar.activation(
                        tanh_g[:], xzg[2][rows, :], tanh, scale=1.0 / XS
                    )
                    nc.scalar.activation(sig_o[:], xzg[3][rows, :], sig, scale=1.0 / XS)

                    t1 = gp.tile([B, HIDDEN], BF, tag="t1", name="t1")
                    t2 = gp.tile([B, HIDDEN], BF, tag="t2", name="t2")
                    c_new = gp.tile([B, HIDDEN], BF, tag="c_new", name="c_new")
                    nc.vector.tensor_mul(t1[:], sig_f[:], c_sb[:])
                    nc.vector.tensor_mul(t2[:], sig_i[:], tanh_g[:])
                    nc.vector.tensor_add(c_new[:], t1[:], t2[:])

                    m_bc = mask_sb[:, t : t + 1].to_broadcast([B, HIDDEN])
                    # masked (token==0) steps carry previous state; in-place blend
                    nc.vector.copy_predicated(c_sb[:], m_bc, c_new[:])

                    # h path uses pre-mask c_new: masked rows discard h_new anyway
                    tanh_c = gp.tile([B, HIDDEN], BF, tag="tanh_c", name="tanh_c")
                    nc.scalar.activation(tanh_c[:], c_new[:], tanh)
                    h_new = gp.tile([B, HIDDEN], BF, tag="h_new", name="h_new")
                    nc.vector.tensor_mul(h_new[:], sig_o[:], tanh_c[:])
                    nc.vector.copy_predicated(h_sb[:], m_bc, h_new[:])

                    transpose_h(t)

                # filler work at very low priority: drips into PE idle gaps
                with low_priority(tc):
                    if g + 1 < NT:
                        xg_cur = stage_xg(embT_nxt)
                        if g + 2 < NT:
                            embT_nxt = load_embT(g + 2)
                    if g >= 1:
                        logits_group(g - 1)

            with low_priority(tc):
                logits_group(NT - 1)

    return nc


def _get_program() -> bass.Bass:
    if "nc" not in _CACHE:
        _CACHE["nc"] = _build_program()
    return _CACHE["nc"]


def prep_in_maps(inputs) -> list:
    import ml_dtypes

    bf16 = ml_dtypes.bfloat16
    fp8 = ml_dtypes.float8_e4m3
    tok = np.asarray(inputs["target_tokens"])
    ctx = np.asarray(inputs["context"], dtype=np.float32)
    emb_table = np.asarray(inputs["emb_table"], np.float32)
    w_h = np.asarray(inputs["W_h"], np.float32)
    w_out = np.asarray(inputs["W_out"], np.float32)
    b_out = np.asarray(inputs["b_out"], np.float32)

    mask = (tok != 0).astype(np.uint8)  # [B, S]
    tok_t = tok.T.reshape(-1).astype(np.int64)  # t*B + b token order
    emb_t = np.ascontiguousarray((emb_table[tok_t].T * XS).astype(bf16))  # [E, T]
    ctx_t = np.ascontiguousarray(ctx.T.astype(bf16))  # [CTX, B]

    w_h3 = np.clip(w_h * WS, -240.0, 240.0)  # [512, 2048] scaled
    w_h3 = np.ascontiguousarray(
        w_h3.reshape(NK, P, G4).transpose(1, 0, 2).reshape(P, NK * G4).astype(fp8)
    )

    shared = {
        "context_t": ctx_t,
        "emb_t": emb_t,
        "w_ih": np.ascontiguousarray(np.asarray(inputs["W_ih"]).astype(bf16)),
        "w_ic": np.ascontiguousarray(np.asarray(inputs["W_ic"]).astype(bf16)),
        "w_x": np.ascontiguousarray(np.asarray(inputs["W_x"]).astype(bf16)),
        "w_h3": w_h3,
        "b_g": np.ascontiguousarray((np.asarray(inputs["b"]) * XS).astype(bf16)),
        "b_ih": np.ascontiguousarray(np.asarray(inputs["b_ih"]).astype(bf16)),
        "b_ic": np.ascontiguousarray(np.asarray(inputs["b_ic"]).astype(bf16)),
        "maskf": np.ascontiguousarray(mask),
    }
    in_maps = []
    for j in range(NCORES):
        m = dict(shared)
        m["w_out"] = np.ascontiguousarray(
            w_out[:, j * VSH : (j + 1) * VSH].astype(bf16)
        )
        m["b_out"] = np.ascontiguousarray(b_out[j * VSH : (j + 1) * VSH].astype(bf16))
        in_maps.append(m)
    return in_maps


def kernel(**inputs: np.ndarray) -> np.ndarray:
    in_maps = prep_in_maps(inputs)
    nc = _get_program()
    if not nc.is_finalized():
        nc.finalize()

    import os

    trace = bool(os.environ.get("CAPDEC_TRACE"))
    kw = {}
    if trace:
        kw["trace"] = True
        tdir = os.environ.get("CAPDEC_TRACE_DIR")
        if tdir:
            os.makedirs(tdir, exist_ok=True)
            kw["tmpdir"] = tdir
    bkr = run_bass_kernel_spmd(nc, in_maps, list(range(NCORES)), **kw)
    _CACHE["last_results"] = bkr
    res = bkr.results
    parts = [
        np.asarray(res[j]["logits"]).astype(np.float32).reshape(S, B, VSH)
        for j in range(NCORES)
    ]
    full = np.concatenate(parts, axis=-1)  # [S, B, VOCAB]
    return np.ascontiguousarray(full.transpose(1, 0, 2))


# revision 16
# speedup vs baseline: 1.1073x; 1.0332x over previous
"""Trainium2 Bass kernel for nn_CaptionDecoder (embedding -> masked LSTM -> vocab projection).

Sharding: the LSTM (B=32, S=64, H=512) is replicated on all 8 cores; the
vocab dimension of W_out/b_out is sharded 8-way (4000 per core). Each core
emits logits [S*B, 4000] bf16; the host concatenates along vocab -> f32.

Device dataflow per core (pipelined across 16 groups of 4 LSTM steps):
  - emb gathered+transposed on host -> emb_t [E, T] (pre-scaled x2048), streamed
  - xg = emb@W_x + b staged into SBUF (bf16) one group ahead through a small
    PSUM buffer; injected into four per-gate PSUM tiles [128, 512]
  - recurrence h_{t-1} @ W_h runs in fp8 DoubleRow (2x PE throughput): the
    transposed h ring is kept in fp8 (h pre-scaled x64), W_h in fp8 (x32),
    so gate pre-activations come out x2048 and the ScalarE activation's free
    input scale (1/2048) undoes it exactly
  - state update: bf16 c/h; c is blended in place with copy_predicated
    (Keras mask_zero), h via two scalar_tensor_tensor ops folding the mask
    and the x64 h scale
  - a second bf16 ring feeds the logits matmuls (keeps logits free of fp8
    input noise); W_out is pre-divided by 64 on host to undo the h scale
  - logits: ring block [128,128] stationary, W_out streamed, bias via K=1
    ones matmul, ScalarE/DVE copy to SBUF bf16, DMA out
  - filler work (xg staging, logits) is emitted at very low scheduler
    priority so it drips into PE idle gaps instead of delaying the chain;
    resident weight DMAs ride the idle GpSimd queue so they don't block the
    h0/c0 init path at startup.
"""

import sys
from contextlib import contextmanager

import numpy as np

if "/opt/trn_rl_repo" not in sys.path:
    sys.path.insert(0, "/opt/trn_rl_repo")

import concourse.bass as bass
import concourse.bacc as bacc
import concourse.mybir as mybir
import concourse.tile as tile
from concourse.bass_utils import run_bass_kernel_spmd
from concourse.masks import make_identity

VOCAB, EMBED, HIDDEN, CTX = 32000, 512, 512, 2048
B, S = 32, 64
G4 = 4 * HIDDEN  # 2048 gate width
NCORES = 8
VSH = VOCAB // NCORES  # 4000 vocab per core
P = 128
T = S * B  # 2048 tokens, t-major (tok = t*B + b)
NT = T // P  # 16 token tiles / groups
NK = HIDDEN // P  # 4 k-chunks over hidden/embed
NKC = CTX // P  # 16 k-chunks over context
NV = 8  # vocab slices per core
VS = VSH // NV  # 500 wide each
F32 = mybir.dt.float32
BF = mybir.dt.bfloat16
F8 = mybir.dt.float8e4

WS = 32.0  # W_h fp8 pre-scale
HS = 64.0  # h fp8 pre-scale
XS = WS * HS  # gate pre-activation scale (undone by activation input scale)

_CACHE: dict = {}

sig = mybir.ActivationFunctionType.Sigmoid
tanh = mybir.ActivationFunctionType.Tanh
MULT = mybir.AluOpType.mult
ADD = mybir.AluOpType.add


@contextmanager
def low_priority(tc, bump=1_000_000):
    """Emit instructions as if issued much later: the scheduler only picks
    them when nothing chain-critical is ready (pure filler work)."""
    p = tc.cur_priority
    tc.cur_priority = p + bump
    try:
        yield
    finally:
        tc.cur_priority = p


def _build_program() -> bass.Bass:
    nc = bacc.Bacc(None)

    ctx_d = nc.declare_dram_parameter("context_t", [CTX, B], BF, isOutput=False)
    embt_d = nc.declare_dram_parameter("emb_t", [EMBED, T], BF, isOutput=False)
    wih_d = nc.declare_dram_parameter("w_ih", [CTX, HIDDEN], BF, isOutput=False)
    wic_d = nc.declare_dram_parameter("w_ic", [CTX, HIDDEN], BF, isOutput=False)
    wx_d = nc.declare_dram_parameter("w_x", [EMBED, G4], BF, isOutput=False)
    wh3_d = nc.declare_dram_parameter("w_h3", [P, NK * G4], F8, isOutput=False)
    bg_d = nc.declare_dram_parameter("b_g", [G4], BF, isOutput=False)
    bih_d = nc.declare_dram_parameter("b_ih", [HIDDEN], BF, isOutput=False)
    bic_d = nc.declare_dram_parameter("b_ic", [HIDDEN], BF, isOutput=False)
    wout_d = nc.declare_dram_parameter("w_out", [HIDDEN, VSH], BF, isOutput=False)
    bout_d = nc.declare_dram_parameter("b_out", [VSH], BF, isOutput=False)
    mask_d = nc.declare_dram_parameter("maskf", [B, S], mybir.dt.uint8, isOutput=False)
    maskt_d = nc.declare_dram_parameter("maskT", [P, T], mybir.dt.uint8, isOutput=False)
    out_d = nc.declare_dram_parameter("logits", [T, VSH], BF, isOutput=True)

    with tile.TileContext(nc) as tc:
        with (
            tc.tile_pool(name="const", bufs=1) as cp,
            tc.tile_pool(name="stream", bufs=2) as sp,
            tc.tile_pool(name="embp", bufs=2) as ep,
            tc.tile_pool(name="xgp", bufs=2) as xp,
            tc.tile_pool(name="gates", bufs=2) as gp,
            tc.tile_pool(name="lout", bufs=3) as lp,
            tc.tile_pool(name="pz", bufs=1, space="PSUM") as pz,
            tc.tile_pool(name="pstage", bufs=1, space="PSUM") as psg,
            tc.tile_pool(name="pa", bufs=2, space="PSUM") as pa,
            tc.tile_pool(name="ptr", bufs=1, space="PSUM") as pt,
        ):
            # ---- resident constants / weights ----
            identb = cp.tile([P, P], BF, tag="identb", name="identb")
            make_identity(nc, identb[:])
            ones1 = cp.tile([1, P], BF, tag="ones1", name="ones1")
            nc.vector.memset(ones1[:], 1.0)

            ctx_sb = cp.tile([P, NKC * B], BF, tag="ctx", name="ctx")
            nc.sync.dma_start(
                out=ctx_sb[:].rearrange("p (k b) -> p k b", b=B),
                in_=ctx_d.rearrange("(k p) b -> p k b", p=P),
            )
            mask_sb = cp.tile([B, S], mybir.dt.uint8, tag="mask", name="mask")
            nc.sync.dma_start(out=mask_sb[:], in_=mask_d[:, :])
            maskt_sb = cp.tile([P, T], mybir.dt.uint8, tag="maskT", name="maskT")
            nc.sync.dma_start(out=maskt_sb[:], in_=maskt_d[:, :])
            bg_sb = cp.tile([1, G4], BF, tag="bg", name="bg")
            nc.sync.dma_start(out=bg_sb[:], in_=bg_d[None, :])
            bout_sb = cp.tile([1, VSH], BF, tag="bout", name="bout")
            nc.sync.dma_start(out=bout_sb[:], in_=bout_d[None, :])
            bih_sb = cp.tile([1, HIDDEN], BF, tag="bih", name="bih")
            nc.sync.dma_start(out=bih_sb[:], in_=bih_d[None, :])
            bic_sb = cp.tile([1, HIDDEN], BF, tag="bic", name="bic")
            nc.sync.dma_start(out=bic_sb[:], in_=bic_d[None, :])

            # resident weights load at low priority so the h0/c0 init path's
            # streaming DMAs win the queue at startup
            wh3_sb = cp.tile([P, NK * G4], F8, tag="wh3", name="wh3")
            wx_sb = []
            wout_sb = []
            with low_priority(tc):
                nc.sync.dma_start(out=wh3_sb[:], in_=wh3_d[:, :])
                for k in range(NK):
                    t_wx = cp.tile([P, G4], BF, tag=f"wx{k}", name=f"wx{k}")
                    nc.sync.dma_start(out=t_wx[:], in_=wx_d[k * P : (k + 1) * P, :])
                    wx_sb.append(t_wx)
                    t_wo = cp.tile([P, VSH], BF, tag=f"wout{k}", name=f"wout{k}")
                    nc.sync.dma_start(
                        out=t_wo[:], in_=wout_d[k * P : (k + 1) * P, :]
                    )
                    wout_sb.append(t_wo)
            wh3_v = wh3_sb[:].rearrange("p (o n) -> p o n", o=NK)

            # ---- embedding tiles (prefetched), staged xg in SBUF ----
            def load_embT(g):
                ts = []
                for k in range(NK):
                    et = ep.tile([P, P], BF, tag=f"embT{k}", name=f"embT{k}")
                    nc.sync.dma_start(
                        out=et[:],
                        in_=embt_d[k * P : (k + 1) * P, g * P : (g + 1) * P],
                    )
                    ts.append(et)
                return ts

            def stage_xg(embT):
                """xg = emb @ W_x + b for one group -> SBUF bf16 [128, 2048]."""
                xg = xp.tile([P, G4], BF, tag="xg", name="xg")
                for n in range(4):
                    ns = slice(n * HIDDEN, (n + 1) * HIDDEN)
                    ps_t = psg.tile([P, HIDDEN], F32, tag="xs", name="ps_t")
                    for k in range(NK):
                        nc.tensor.matmul(
                            out=ps_t[:],
                            lhsT=(embT[k][:]),
                            rhs=(wx_sb[k][:, ns]),
                            start=(k == 0),
                            stop=False,
                        )
                    nc.tensor.matmul(
                        out=ps_t[:],
                        lhsT=(ones1[:1, :]),
                        rhs=(bg_sb[:1, ns]),
                        start=False,
                        stop=True,
                    )
                    nc.vector.tensor_copy(xg[:, ns], ps_t[:])
                return xg

            # ---- state tiles ----
            h_sb = cp.tile([B, HIDDEN], BF, tag="h", name="h")
            c_sb = cp.tile([B, HIDDEN], BF, tag="c", name="c")

            # h transpose rings: slot(t) = t % 8, cols (k*8 + slot)*32
            ring8 = cp.tile([P, NK * 8 * B], F8, tag="ring8", name="ring8")
            ringb = cp.tile([P, NK * 8 * B], BF, tag="ringb", name="ringb")

            ring8_v = ring8[:].rearrange("p (k s c) -> p k s c", k=NK, s=8)
            ringb_v = ringb[:].rearrange("p (k s c) -> p k s c", k=NK, s=8)

            def transpose_h0(t):
                """PE-transpose h0 [32,512] into both rings' slot t%8 (x64)."""
                slot = t % 8
                tp = pt.tile([P, 2 * P], BF, tag="tp", name="tp")
                for k in range(NK):
                    nc.tensor.transpose(
                        out=tp[:, k * B : (k + 1) * B],
                        in_=h_sb[:, k * P : (k + 1) * P],
                        identity=identb[:B, :B],
                    )
                srcv = tp[:, :P].rearrange("p (k c) -> p k c", k=NK)
                nc.vector.tensor_scalar_mul(ring8_v[:, :, slot, :], srcv, HS)
                nc.vector.tensor_scalar_mul(ringb_v[:, :, slot, :], srcv, HS)

            embT_cur = load_embT(0)
            embT_nxt = load_embT(1)

            # ---- initial state h0/c0 = tanh(context @ W + b) in gate tiles ----
            xz0_h = pz.tile([P, HIDDEN], F32, tag="xzg0", name="xz0_h")
            xz0_c = pz.tile([P, HIDDEN], F32, tag="xzg1", name="xz0_c")
            for w_dram, b_sb, dst in (
                (wih_d, bih_sb, xz0_h),
                (wic_d, bic_sb, xz0_c),
            ):
                for kc in range(NKC):
                    wt = sp.tile([P, HIDDEN], BF, tag="wstream", name="wstream")
                    nc.sync.dma_start(out=wt[:], in_=w_dram[kc * P : (kc + 1) * P, :])
                    nc.tensor.matmul(
                        out=dst[:B, :],
                        lhsT=(ctx_sb[:, kc * B : (kc + 1) * B]),
                        rhs=(wt[:]),
                        start=(kc == 0),
                        stop=False,
                    )
                nc.tensor.matmul(
                    out=dst[:B, :],
                    lhsT=(ones1[:1, :B]),
                    rhs=(b_sb[:1, :]),
                    start=False,
                    stop=True,
                )
            nc.scalar.activation(h_sb[:], xz0_h[:B, :], tanh)
            nc.scalar.activation(c_sb[:], xz0_c[:B, :], tanh)
            transpose_h0(-1)  # h0 into slot 7

            xg_cur = stage_xg(embT_cur)

            def logits_group(g):
                """Vocab-sharded logits for token tile g from the bf16 ring."""
                half = (g % 2) * 4
                for v in range(NV):
                    vs = slice(v * VS, (v + 1) * VS)
                    pl = pa.tile([P, VS], F32, tag="pl", name="pl")
                    for k in range(NK):
                        cbase = (k * 8 + half) * B
                        nc.tensor.matmul(
                            out=pl[:],
                            lhsT=(ringb[:, cbase : cbase + 4 * B]),
                            rhs=(wout_sb[k][:, vs]),
                            start=(k == 0),
                            stop=False,
                        )
                    nc.tensor.matmul(
                        out=pl[:],
                        lhsT=(ones1[:1, :]),
                        rhs=(bout_sb[:1, vs]),
                        start=False,
                        stop=True,
                    )
                    lo = lp.tile([P, VS], BF, tag="lo", name="lo")
                    nc.scalar.copy(lo[:], pl[:])
                    nc.sync.dma_start(out=out_d[g * P : (g + 1) * P, vs], in_=lo[:])

            # ---- main loop ----
            gate_tags = ["xzg0", "xzg1", "xzg2", "xzg3"]
            for g in range(NT):
                for s in range(4):
                    t = 4 * g + s
                    rows = slice(0, B)
                    slot_prev = (t - 1) % 8

                    # masked-step fallback: pre-copy previous slot into slot t
                    nc.vector.tensor_copy(
                        ring8_v[:, :, t % 8, :], ring8_v[:, :, slot_prev, :]
                    )
                    nc.vector.tensor_copy(
                        ringb_v[:, :, t % 8, :], ringb_v[:, :, slot_prev, :]
                    )

                    # per-step inject: rows 32s of staged xg -> psum rows 0:32
                    # (DoubleRow requires dst partition base 0, so the 4 steps
                    # sequentially reuse the same per-gate psum rows)
                    xzg = []
                    for n in range(4):
                        zt = pz.tile(
                            [P, HIDDEN], F32, tag=gate_tags[n], name=f"xz{n}"
                        )
                        xzg.append(zt)
                    for n in (1, 0, 2, 3):
                        nc.tensor.matmul(
                            out=xzg[n][rows, :],
                            lhsT=(identb[:, B * s : B * (s + 1)]),
                            rhs=(xg_cur[:, n * HIDDEN : (n + 1) * HIDDEN]),
                            start=True,
                            stop=True,
                        )

                    # recurrence: z += (64h) @ (32W_h) in fp8 DoubleRow
                    for n in (1, 0, 2, 3):  # Keras gate order is i,f,g,o
                        ns = slice(n * HIDDEN, (n + 1) * HIDDEN)
                        for j in range(2):
                            nc.tensor.matmul(
                                out=xzg[n][rows, :],
                                lhsT=ring8_v[:, 2 * j : 2 * j + 2, slot_prev, :],
                                rhs=wh3_v[:, 2 * j : 2 * j + 2, ns],
                                perf_mode=mybir.MatmulPerfMode.DoubleRow,
                                start=False,
                                stop=False,
                                skip_group_check=True,
                            )

                    sig_f = gp.tile([B, HIDDEN], BF, tag="sig_f", name="sig_f")
                    sig_i = gp.tile([B, HIDDEN], BF, tag="sig_i", name="sig_i")
                    tanh_g = gp.tile([B, HIDDEN], BF, tag="tanh_g", name="tanh_g")
                    sig_o = gp.tile([B, HIDDEN], BF, tag="sig_o", name="sig_o")
                    nc.scalar.activation(sig_f[:], xzg[1][rows, :], sig, scale=1.0 / XS)
                    nc.scalar.activation(sig_i[:], xzg[0][rows, :], sig, scale=1.0 / XS)
                    nc.scalar.activation(
                        tanh_g[:], xzg[2][rows, :], tanh, scale=1.0 / XS
                    )
                    nc.scalar.activation(sig_o[:], xzg[3][rows, :], sig, scale=1.0 / XS)

                    t1 = gp.tile([B, HIDDEN], BF, tag="t1", name="t1")
                    t2 = gp.tile([B, HIDDEN], BF, tag="t2", name="t2")
                    c_new = gp.tile([B, HIDDEN], BF, tag="c_new", name="c_new")
                    nc.vector.tensor_mul(t1[:], sig_f[:], c_sb[:])
                    nc.vector.tensor_mul(t2[:], sig_i[:], tanh_g[:])
                    nc.vector.tensor_add(c_new[:], t1[:], t2[:])

                    m_bc = mask_sb[:, t : t + 1].to_broadcast([B, HIDDEN])
                    # masked (token==0) steps carry previous state; in-place blend
                    nc.vector.copy_predicated(c_sb[:], m_bc, c_new[:])

                    # h tail in transposed space: h_t = sig_o * tanh(c_new),
                    # computed as 64*h directly into both ring slots
                    tp = pt.tile([P, 2 * P], BF, tag="tp", name="tp")
                    for k in range(NK):
                        nc.tensor.transpose(
                            out=tp[:, k * B : (k + 1) * B],
                            in_=c_new[:, k * P : (k + 1) * P],
                            identity=identb[:B, :B],
                        )
                        nc.tensor.transpose(
                            out=tp[:, P + k * B : P + (k + 1) * B],
                            in_=sig_o[:, k * P : (k + 1) * P],
                            identity=identb[:B, :B],
                        )
                    so_t = gp.tile([P, P], BF, tag="so_t", name="so_t")
                    nc.vector.tensor_copy(so_t[:], tp[:, P : 2 * P])
                    th_t = gp.tile([P, P], BF, tag="th_t", name="th_t")
                    nc.scalar.activation(th_t[:], tp[:, :P], tanh)
                    hv64 = gp.tile([P, P], BF, tag="hv64", name="hv64")
                    nc.vector.scalar_tensor_tensor(
                        out=hv64[:],
                        in0=th_t[:],
                        scalar=HS,
                        in1=so_t[:],
                        op0=MULT,
                        op1=MULT,
                    )
                    slot = t % 8
                    hv64_v = hv64[:].rearrange("p (k c) -> p k c", k=NK)
                    mt_bc = (
                        maskt_sb[:, B * t : B * (t + 1)]
                        .unsqueeze(1)
                        .to_broadcast([P, NK, B])
                    )
                    nc.vector.copy_predicated(ring8_v[:, :, slot, :], mt_bc, hv64_v)
                    nc.vector.copy_predicated(ringb_v[:, :, slot, :], mt_bc, hv64_v)

                # filler work at very low priority: drips into PE idle gaps
                with low_priority(tc):
                    if g + 1 < NT:
                        xg_cur = stage_xg(embT_nxt)
                        if g + 2 < NT:
                            embT_nxt = load_embT(g + 2)
                    if g >= 1:
                        logits_group(g - 1)

            with low_priority(tc):
                logits_group(NT - 1)

    return nc


def _get_program() -> bass.Bass:
    if "nc" not in _CACHE:
        _CACHE["nc"] = _build_program()
    return _CACHE["nc"]


def prep_in_maps(inputs) -> list:
    import ml_dtypes

    bf16 = ml_dtypes.bfloat16
    fp8 = ml_dtypes.float8_e4m3
    tok = np.asarray(inputs["target_tokens"])
    ctx = np.asarray(inputs["context"], dtype=np.float32)
    emb_table = np.asarray(inputs["emb_table"], np.float32)
    w_h = np.asarray(inputs["W_h"], np.float32)
    w_out = np.asarray(inputs["W_out"], np.float32)
    b_out = np.asarray(inputs["b_out"], np.float32)

    mask = (tok != 0).astype(np.uint8)  # [B, S]
    tok_t = tok.T.reshape(-1).astype(np.int64)  # t*B + b token order
    emb_t = np.ascontiguousarray((emb_table[tok_t].T * XS).astype(bf16))  # [E, T]
    ctx_t = np.ascontiguousarray(ctx.T.astype(bf16))  # [CTX, B]

    w_h3 = np.clip(w_h * WS, -240.0, 240.0)  # [512, 2048] scaled
    w_h3 = np.ascontiguousarray(
        w_h3.reshape(NK, P, G4).transpose(1, 0, 2).reshape(P, NK * G4).astype(fp8)
    )

    shared = {
        "context_t": ctx_t,
        "emb_t": emb_t,
        "w_ih": np.ascontiguousarray(np.asarray(inputs["W_ih"]).astype(bf16)),
        "w_ic": np.ascontiguousarray(np.asarray(inputs["W_ic"]).astype(bf16)),
        "w_x": np.ascontiguousarray(np.asarray(inputs["W_x"]).astype(bf16)),
        "w_h3": w_h3,
        "b_g": np.ascontiguousarray((np.asarray(inputs["b"]) * XS).astype(bf16)),
        "b_ih": np.ascontiguousarray(np.asarray(inputs["b_ih"]).astype(bf16)),
        "b_ic": np.ascontiguousarray(np.asarray(inputs["b_ic"]).astype(bf16)),
        "maskf": np.ascontiguousarray(mask),
        "maskT": np.ascontiguousarray(
            np.broadcast_to(mask.T.reshape(1, -1), (P, T)).copy()
        ),
    }
    in_maps = []
    for j in range(NCORES):
        m = dict(shared)
        m["w_out"] = np.ascontiguousarray(
            (w_out[:, j * VSH : (j + 1) * VSH] / HS).astype(bf16)
        )
        m["b_out"] = np.ascontiguousarray(b_out[j * VSH : (j + 1) * VSH].astype(bf16))
        in_maps.append(m)
    return in_maps


def kernel(**inputs: np.ndarray) -> np.ndarray:
    in_maps = prep_in_maps(inputs)
    nc = _get_program()
    if not nc.is_finalized():
        nc.finalize()

    import os

    trace = bool(os.environ.get("CAPDEC_TRACE"))
    kw = {}
    if trace:
        kw["trace"] = True
        tdir = os.environ.get("CAPDEC_TRACE_DIR")
        if tdir:
            os.makedirs(tdir, exist_ok=True)
            kw["tmpdir"] = tdir
    bkr = run_bass_kernel_spmd(nc, in_maps, list(range(NCORES)), **kw)
    _CACHE["last_results"] = bkr
    res = bkr.results
    parts = [
        np.asarray(res[j]["logits"]).astype(np.float32).reshape(S, B, VSH)
        for j in range(NCORES)
    ]
    full = np.concatenate(parts, axis=-1)  # [S, B, VOCAB]
    return np.ascontiguousarray(full.transpose(1, 0, 2))


# revision 17
# speedup vs baseline: 1.1550x; 1.0431x over previous
"""Trainium2 Bass kernel for nn_CaptionDecoder (embedding -> masked LSTM -> vocab projection).

Sharding: the LSTM (B=32, S=64, H=512) is replicated on all 8 cores; the
vocab dimension of W_out/b_out is sharded 8-way (4000 per core). Each core
emits logits [S*B, 4000] bf16; the host concatenates along vocab -> f32.

Device dataflow per core (pipelined across 16 groups of 4 LSTM steps):
  - emb gathered+transposed on host -> emb_t [E, T] (pre-scaled x2048), streamed
  - xg = emb@W_x + b staged into SBUF (bf16) one group ahead through a small
    PSUM buffer; injected into four per-gate PSUM tiles [128, 512]
  - recurrence h_{t-1} @ W_h runs in fp8 DoubleRow (2x PE throughput): the
    transposed h ring is kept in fp8 (h pre-scaled x64), W_h in fp8 (x32),
    so gate pre-activations come out x2048 and the ScalarE activation's free
    input scale (1/2048) undoes it exactly
  - state update: bf16 c/h; c is blended in place with copy_predicated
    (Keras mask_zero), h via two scalar_tensor_tensor ops folding the mask
    and the x64 h scale
  - a second bf16 ring feeds the logits matmuls (keeps logits free of fp8
    input noise); W_out is pre-divided by 64 on host to undo the h scale
  - logits: ring block [128,128] stationary, W_out streamed, bias via K=1
    ones matmul, ScalarE/DVE copy to SBUF bf16, DMA out
  - filler work (xg staging, logits) is emitted at very low scheduler
    priority so it drips into PE idle gaps instead of delaying the chain;
    resident weight DMAs ride the idle GpSimd queue so they don't block the
    h0/c0 init path at startup.
"""

import sys
from contextlib import contextmanager

import numpy as np

if "/opt/trn_rl_repo" not in sys.path:
    sys.path.insert(0, "/opt/trn_rl_repo")

import concourse.bass as bass
import concourse.bacc as bacc
import concourse.mybir as mybir
import concourse.tile as tile
from concourse.bass_utils import run_bass_kernel_spmd
from concourse.masks import make_identity

VOCAB, EMBED, HIDDEN, CTX = 32000, 512, 512, 2048
B, S = 32, 64
G4 = 4 * HIDDEN  # 2048 gate width
NCORES = 8
VSH = VOCAB // NCORES  # 4000 vocab per core
P = 128
T = S * B  # 2048 tokens, t-major (tok = t*B + b)
NT = T // P  # 16 token tiles / groups
NK = HIDDEN // P  # 4 k-chunks over hidden/embed
NKC = CTX // P  # 16 k-chunks over context
NV = 8  # vocab slices per core
VS = VSH // NV  # 500 wide each
F32 = mybir.dt.float32
BF = mybir.dt.bfloat16
F8 = mybir.dt.float8e4

WS = 32.0  # W_h fp8 pre-scale
HS = 64.0  # h fp8 pre-scale
XS = WS * HS  # gate pre-activation scale (undone by activation input scale)

_CACHE: dict = {}

sig = mybir.ActivationFunctionType.Sigmoid
tanh = mybir.ActivationFunctionType.Tanh
MULT = mybir.AluOpType.mult
ADD = mybir.AluOpType.add


@contextmanager
def low_priority(tc, bump=1_000_000):
    """Emit instructions as if issued much later: the scheduler only picks
    them when nothing chain-critical is ready (pure filler work)."""
    p = tc.cur_priority
    tc.cur_priority = p + bump
    try:
        yield
    finally:
        tc.cur_priority = p


def _build_program() -> bass.Bass:
    nc = bacc.Bacc(None)

    ctx_d = nc.declare_dram_parameter("context_t", [CTX, B], BF, isOutput=False)
    embt_d = nc.declare_dram_parameter("emb_t", [EMBED, T], BF, isOutput=False)
    wih_d = nc.declare_dram_parameter("w_ih", [CTX, HIDDEN], BF, isOutput=False)
    wic_d = nc.declare_dram_parameter("w_ic", [CTX, HIDDEN], BF, isOutput=False)
    wx_d = nc.declare_dram_parameter("w_x", [EMBED, G4], BF, isOutput=False)
    wh3_d = nc.declare_dram_parameter("w_h3", [P, NK * G4], F8, isOutput=False)
    bg_d = nc.declare_dram_parameter("b_g", [G4], BF, isOutput=False)
    bih_d = nc.declare_dram_parameter("b_ih", [HIDDEN], BF, isOutput=False)
    bic_d = nc.declare_dram_parameter("b_ic", [HIDDEN], BF, isOutput=False)
    wout_d = nc.declare_dram_parameter("w_out", [HIDDEN, VSH], BF, isOutput=False)
    bout_d = nc.declare_dram_parameter("b_out", [VSH], BF, isOutput=False)
    mask_d = nc.declare_dram_parameter("maskf", [B, S], mybir.dt.uint8, isOutput=False)
    maskt_d = nc.declare_dram_parameter("maskT", [P, T], mybir.dt.uint8, isOutput=False)
    out_d = nc.declare_dram_parameter("logits", [T, VSH], BF, isOutput=True)

    with tile.TileContext(nc) as tc:
        with (
            tc.tile_pool(name="const", bufs=1) as cp,
            tc.tile_pool(name="stream", bufs=2) as sp,
            tc.tile_pool(name="embp", bufs=2) as ep,
            tc.tile_pool(name="xgp", bufs=2) as xp,
            tc.tile_pool(name="gates", bufs=2) as gp,
            tc.tile_pool(name="lout", bufs=3) as lp,
            tc.tile_pool(name="pz", bufs=1, space="PSUM") as pz,
            tc.tile_pool(name="pstage", bufs=2, space="PSUM") as psg,
            tc.tile_pool(name="pa", bufs=1, space="PSUM") as pa,
            tc.tile_pool(name="ptr", bufs=1, space="PSUM") as pt,
        ):
            # ---- resident constants / weights ----
            identb = cp.tile([P, P], BF, tag="identb", name="identb")
            make_identity(nc, identb[:])
            ones1 = cp.tile([1, P], BF, tag="ones1", name="ones1")
            nc.vector.memset(ones1[:], 1.0)

            ctx_sb = cp.tile([P, NKC * B], BF, tag="ctx", name="ctx")
            nc.sync.dma_start(
                out=ctx_sb[:].rearrange("p (k b) -> p k b", b=B),
                in_=ctx_d.rearrange("(k p) b -> p k b", p=P),
            )
            mask_sb = cp.tile([B, S], mybir.dt.uint8, tag="mask", name="mask")
            nc.sync.dma_start(out=mask_sb[:], in_=mask_d[:, :])
            maskt_sb = cp.tile([P, T], mybir.dt.uint8, tag="maskT", name="maskT")
            nc.sync.dma_start(out=maskt_sb[:], in_=maskt_d[:, :])
            bg_sb = cp.tile([1, G4], BF, tag="bg", name="bg")
            nc.sync.dma_start(out=bg_sb[:], in_=bg_d[None, :])
            bout_sb = cp.tile([1, VSH], BF, tag="bout", name="bout")
            nc.sync.dma_start(out=bout_sb[:], in_=bout_d[None, :])
            bih_sb = cp.tile([1, HIDDEN], BF, tag="bih", name="bih")
            nc.sync.dma_start(out=bih_sb[:], in_=bih_d[None, :])
            bic_sb = cp.tile([1, HIDDEN], BF, tag="bic", name="bic")
            nc.sync.dma_start(out=bic_sb[:], in_=bic_d[None, :])

            # resident weights load at low priority so the h0/c0 init path's
            # streaming DMAs win the queue at startup
            wh3_sb = cp.tile([P, NK * G4], F8, tag="wh3", name="wh3")
            wx_sb = []
            wout_sb = []
            with low_priority(tc):
                nc.sync.dma_start(out=wh3_sb[:], in_=wh3_d[:, :])
                for k in range(NK):
                    t_wx = cp.tile([P, G4], BF, tag=f"wx{k}", name=f"wx{k}")
                    nc.sync.dma_start(out=t_wx[:], in_=wx_d[k * P : (k + 1) * P, :])
                    wx_sb.append(t_wx)
                for k in range(NK):
                    t_wo = cp.tile([P, VSH], BF, tag=f"wout{k}", name=f"wout{k}")
                    nc.sync.dma_start(
                        out=t_wo[:], in_=wout_d[k * P : (k + 1) * P, :]
                    )
                    wout_sb.append(t_wo)
            wh3_v = wh3_sb[:].rearrange("p (o n) -> p o n", o=NK)

            # ---- embedding tiles (prefetched), staged xg in SBUF ----
            def load_embT(g):
                ts = []
                for k in range(NK):
                    et = ep.tile([P, P], BF, tag=f"embT{k}", name=f"embT{k}")
                    nc.sync.dma_start(
                        out=et[:],
                        in_=embt_d[k * P : (k + 1) * P, g * P : (g + 1) * P],
                    )
                    ts.append(et)
                return ts

            def stage_xg(embT):
                """xg = emb @ W_x + b for one group -> SBUF bf16 [128, 2048]."""
                xg = xp.tile([P, G4], BF, tag="xg", name="xg")
                for n in range(4):
                    ns = slice(n * HIDDEN, (n + 1) * HIDDEN)
                    ps_t = psg.tile([P, HIDDEN], F32, tag="xs", name="ps_t")
                    for k in range(NK):
                        nc.tensor.matmul(
                            out=ps_t[:],
                            lhsT=(embT[k][:]),
                            rhs=(wx_sb[k][:, ns]),
                            start=(k == 0),
                            stop=False,
                        )
                    nc.tensor.matmul(
                        out=ps_t[:],
                        lhsT=(ones1[:1, :]),
                        rhs=(bg_sb[:1, ns]),
                        start=False,
                        stop=True,
                    )
                    nc.vector.tensor_copy(xg[:, ns], ps_t[:])
                return xg

            # ---- state tiles ----
            h_sb = cp.tile([B, HIDDEN], BF, tag="h", name="h")
            c_sb = cp.tile([B, HIDDEN], BF, tag="c", name="c")

            # h transpose rings: slot(t) = t % 8, cols (k*8 + slot)*32
            ring8 = cp.tile([P, NK * 8 * B], F8, tag="ring8", name="ring8")
            ringb = cp.tile([P, NK * 8 * B], BF, tag="ringb", name="ringb")

            ring8_v = ring8[:].rearrange("p (k s c) -> p k s c", k=NK, s=8)
            ringb_v = ringb[:].rearrange("p (k s c) -> p k s c", k=NK, s=8)

            def transpose_h0(t):
                """PE-transpose h0 [32,512] into both rings' slot t%8 (x64)."""
                slot = t % 8
                tp = pt.tile([P, 2 * P], BF, tag="tp", name="tp")
                for k in range(NK):
                    nc.tensor.transpose(
                        out=tp[:, k * B : (k + 1) * B],
                        in_=h_sb[:, k * P : (k + 1) * P],
                        identity=identb[:B, :B],
                    )
                srcv = tp[:, :P].rearrange("p (k c) -> p k c", k=NK)
                nc.vector.tensor_scalar_mul(ring8_v[:, :, slot, :], srcv, HS)
                nc.vector.tensor_scalar_mul(ringb_v[:, :, slot, :], srcv, HS)

            embT_cur = load_embT(0)
            embT_nxt = load_embT(1)

            # ---- initial state h0/c0 = tanh(context @ W + b) in gate tiles ----
            xz0_h = pz.tile([P, HIDDEN], F32, tag="xzg0", name="xz0_h")
            xz0_c = pz.tile([P, HIDDEN], F32, tag="xzg1", name="xz0_c")
            for w_dram, b_sb, dst in (
                (wih_d, bih_sb, xz0_h),
                (wic_d, bic_sb, xz0_c),
            ):
                for kc in range(NKC):
                    wt = sp.tile([P, HIDDEN], BF, tag="wstream", name="wstream")
                    nc.sync.dma_start(out=wt[:], in_=w_dram[kc * P : (kc + 1) * P, :])
                    nc.tensor.matmul(
                        out=dst[:B, :],
                        lhsT=(ctx_sb[:, kc * B : (kc + 1) * B]),
                        rhs=(wt[:]),
                        start=(kc == 0),
                        stop=False,
                    )
                nc.tensor.matmul(
                    out=dst[:B, :],
                    lhsT=(ones1[:1, :B]),
                    rhs=(b_sb[:1, :]),
                    start=False,
                    stop=True,
                )
            nc.scalar.activation(h_sb[:], xz0_h[:B, :], tanh)
            nc.scalar.activation(c_sb[:], xz0_c[:B, :], tanh)
            transpose_h0(-1)  # h0 into slot 7

            xg_cur = stage_xg(embT_cur)

            def logits_group(g):
                """Vocab-sharded logits for token tile g from the bf16 ring."""
                half = (g % 2) * 4
                for v in range(NV):
                    vs = slice(v * VS, (v + 1) * VS)
                    pl = pa.tile([P, VS], F32, tag="pl", name="pl")
                    for k in range(NK):
                        cbase = (k * 8 + half) * B
                        nc.tensor.matmul(
                            out=pl[:],
                            lhsT=(ringb[:, cbase : cbase + 4 * B]),
                            rhs=(wout_sb[k][:, vs]),
                            start=(k == 0),
                            stop=False,
                        )
                    nc.tensor.matmul(
                        out=pl[:],
                        lhsT=(ones1[:1, :]),
                        rhs=(bout_sb[:1, vs]),
                        start=False,
                        stop=True,
                    )
                    lo = lp.tile([P, VS], BF, tag="lo", name="lo")
                    nc.scalar.copy(lo[:], pl[:])
                    nc.sync.dma_start(out=out_d[g * P : (g + 1) * P, vs], in_=lo[:])

            # ---- main loop ----
            gate_tags = ["xzg0", "xzg1", "xzg2", "xzg3"]
            for g in range(NT):
                for s in range(4):
                    t = 4 * g + s
                    rows = slice(0, B)
                    slot_prev = (t - 1) % 8

                    # masked-step fallback: pre-copy previous slot into slot t
                    nc.vector.tensor_copy(
                        ring8_v[:, :, t % 8, :], ring8_v[:, :, slot_prev, :]
                    )
                    nc.vector.tensor_copy(
                        ringb_v[:, :, t % 8, :], ringb_v[:, :, slot_prev, :]
                    )

                    # per-step inject: rows 32s of staged xg -> psum rows 0:32
                    # (DoubleRow requires dst partition base 0, so the 4 steps
                    # sequentially reuse the same per-gate psum rows)
                    xzg = []
                    for n in range(4):
                        zt = pz.tile(
                            [P, HIDDEN], F32, tag=gate_tags[n], name=f"xz{n}"
                        )
                        xzg.append(zt)
                    for n in (1, 0, 2, 3):
                        nc.tensor.matmul(
                            out=xzg[n][rows, :],
                            lhsT=(identb[:, B * s : B * (s + 1)]),
                            rhs=(xg_cur[:, n * HIDDEN : (n + 1) * HIDDEN]),
                            start=True,
                            stop=True,
                        )

                    # recurrence: z += (64h) @ (32W_h) in fp8 DoubleRow
                    for n in (1, 0, 2, 3):  # Keras gate order is i,f,g,o
                        ns = slice(n * HIDDEN, (n + 1) * HIDDEN)
                        for j in range(2):
                            nc.tensor.matmul(
                                out=xzg[n][rows, :],
                                lhsT=ring8_v[:, 2 * j : 2 * j + 2, slot_prev, :],
                                rhs=wh3_v[:, 2 * j : 2 * j + 2, ns],
                                perf_mode=mybir.MatmulPerfMode.DoubleRow,
                                start=False,
                                stop=False,
                                skip_group_check=True,
                            )

                    sig_f = gp.tile([B, HIDDEN], BF, tag="sig_f", name="sig_f")
                    sig_i = gp.tile([B, HIDDEN], BF, tag="sig_i", name="sig_i")
                    tanh_g = gp.tile([B, HIDDEN], BF, tag="tanh_g", name="tanh_g")
                    sig_o = gp.tile([B, HIDDEN], BF, tag="sig_o", name="sig_o")
                    nc.scalar.activation(sig_f[:], xzg[1][rows, :], sig, scale=1.0 / XS)
                    nc.scalar.activation(sig_i[:], xzg[0][rows, :], sig, scale=1.0 / XS)
                    nc.scalar.activation(
                        tanh_g[:], xzg[2][rows, :], tanh, scale=1.0 / XS
                    )
                    nc.scalar.activation(sig_o[:], xzg[3][rows, :], sig, scale=1.0 / XS)

                    t1 = gp.tile([B, HIDDEN], BF, tag="t1", name="t1")
                    t2 = gp.tile([B, HIDDEN], BF, tag="t2", name="t2")
                    c_new = gp.tile([B, HIDDEN], BF, tag="c_new", name="c_new")
                    nc.vector.tensor_mul(t1[:], sig_f[:], c_sb[:])
                    nc.vector.tensor_mul(t2[:], sig_i[:], tanh_g[:])
                    nc.vector.tensor_add(c_new[:], t1[:], t2[:])

                    m_bc = mask_sb[:, t : t + 1].to_broadcast([B, HIDDEN])
                    # masked (token==0) steps carry previous state; in-place blend
                    nc.vector.copy_predicated(c_sb[:], m_bc, c_new[:])

                    # h tail in transposed space: h_t = sig_o * tanh(c_new),
                    # computed as 64*h directly into both ring slots
                    tp = pt.tile([P, 2 * P], BF, tag="tp", name="tp")
                    for k in range(NK):
                        nc.tensor.transpose(
                            out=tp[:, k * B : (k + 1) * B],
                            in_=c_new[:, k * P : (k + 1) * P],
                            identity=identb[:B, :B],
                        )
                        nc.tensor.transpose(
                            out=tp[:, P + k * B : P + (k + 1) * B],
                            in_=sig_o[:, k * P : (k + 1) * P],
                            identity=identb[:B, :B],
                        )
                    th_t = gp.tile([P, P], BF, tag="th_t", name="th_t")
                    nc.scalar.activation(th_t[:], tp[:, :P], tanh)
                    hv64 = gp.tile([P, P], BF, tag="hv64", name="hv64")
                    nc.vector.scalar_tensor_tensor(
                        out=hv64[:],
                        in0=th_t[:],
                        scalar=HS,
                        in1=tp[:, P : 2 * P],
                        op0=MULT,
                        op1=MULT,
                    )
                    slot = t % 8
                    hv64_v = hv64[:].rearrange("p (k c) -> p k c", k=NK)
                    mt_bc = (
                        maskt_sb[:, B * t : B * (t + 1)]
                        .unsqueeze(1)
                        .to_broadcast([P, NK, B])
                    )
                    nc.vector.copy_predicated(ring8_v[:, :, slot, :], mt_bc, hv64_v)
                    nc.vector.copy_predicated(ringb_v[:, :, slot, :], mt_bc, hv64_v)

                # filler work at very low priority: drips into PE idle gaps
                with low_priority(tc):
                    if g + 1 < NT:
                        xg_cur = stage_xg(embT_nxt)
                        if g + 2 < NT:
                            embT_nxt = load_embT(g + 2)
                    if g >= 1:
                        logits_group(g - 1)

            with low_priority(tc):
                logits_group(NT - 1)

    return nc


def _get_program() -> bass.Bass:
    if "nc" not in _CACHE:
        _CACHE["nc"] = _build_program()
    return _CACHE["nc"]


def prep_in_maps(inputs) -> list:
    import ml_dtypes

    bf16 = ml_dtypes.bfloat16
    fp8 = ml_dtypes.float8_e4m3
    tok = np.asarray(inputs["target_tokens"])
    ctx = np.asarray(inputs["context"], dtype=np.float32)
    emb_table = np.asarray(inputs["emb_table"], np.float32)
    w_h = np.asarray(inputs["W_h"], np.float32)
    w_out = np.asarray(inputs["W_out"], np.float32)
    b_out = np.asarray(inputs["b_out"], np.float32)

    mask = (tok != 0).astype(np.uint8)  # [B, S]
    tok_t = tok.T.reshape(-1).astype(np.int64)  # t*B + b token order
    emb_t = np.ascontiguousarray((emb_table[tok_t].T * XS).astype(bf16))  # [E, T]
    ctx_t = np.ascontiguousarray(ctx.T.astype(bf16))  # [CTX, B]

    w_h3 = np.clip(w_h * WS, -240.0, 240.0)  # [512, 2048] scaled
    w_h3 = np.ascontiguousarray(
        w_h3.reshape(NK, P, G4).transpose(1, 0, 2).reshape(P, NK * G4).astype(fp8)
    )

    shared = {
        "context_t": ctx_t,
        "emb_t": emb_t,
        "w_ih": np.ascontiguousarray(np.asarray(inputs["W_ih"]).astype(bf16)),
        "w_ic": np.ascontiguousarray(np.asarray(inputs["W_ic"]).astype(bf16)),
        "w_x": np.ascontiguousarray(np.asarray(inputs["W_x"]).astype(bf16)),
        "w_h3": w_h3,
        "b_g": np.ascontiguousarray((np.asarray(inputs["b"]) * XS).astype(bf16)),
        "b_ih": np.ascontiguousarray(np.asarray(inputs["b_ih"]).astype(bf16)),
        "b_ic": np.ascontiguousarray(np.asarray(inputs["b_ic"]).astype(bf16)),
        "maskf": np.ascontiguousarray(mask),
        "maskT": np.ascontiguousarray(
            np.broadcast_to(mask.T.reshape(1, -1), (P, T)).copy()
        ),
    }
    in_maps = []
    for j in range(NCORES):
        m = dict(shared)
        m["w_out"] = np.ascontiguousarray(
            (w_out[:, j * VSH : (j + 1) * VSH] / HS).astype(bf16)
        )
        m["b_out"] = np.ascontiguousarray(b_out[j * VSH : (j + 1) * VSH].astype(bf16))
        in_maps.append(m)
    return in_maps


def kernel(**inputs: np.ndarray) -> np.ndarray:
    in_maps = prep_in_maps(inputs)
    nc = _get_program()
    if not nc.is_finalized():
        nc.finalize()

    import os

    trace = bool(os.environ.get("CAPDEC_TRACE"))
    kw = {}
    if trace:
        kw["trace"] = True
        tdir = os.environ.get("CAPDEC_TRACE_DIR")
        if tdir:
            os.makedirs(tdir, exist_ok=True)
            kw["tmpdir"] = tdir
    bkr = run_bass_kernel_spmd(nc, in_maps, list(range(NCORES)), **kw)
    _CACHE["last_results"] = bkr
    res = bkr.results
    parts = [
        np.asarray(res[j]["logits"]).astype(np.float32).reshape(S, B, VSH)
        for j in range(NCORES)
    ]
    full = np.concatenate(parts, axis=-1)  # [S, B, VOCAB]
    return np.ascontiguousarray(full.transpose(1, 0, 2))
